# revision 47
# baseline (speedup 1.0000x reference)
"""Multi-head self-attention (B=2, L=2048, D=1024, H=16) on 8 Trainium2 cores.

Sharding: batch x head-group. Core c handles batch c//4 and heads
(c%4)*4 .. (c%4)*4+4. Each core computes Q/K/V projections for its 4 heads,
attention, and a partial out-projection (row-slice of W_out); the host sums
the 4 partials per batch and adds b_out.

The device pipeline runs in fp16 (fp32 PSUM accumulation everywhere): fp16
matmuls stream at 1 cycle/row on the PE vs 2 for fp32r, and 2-byte dtypes get
the XBAR DMA-transpose path so x is transposed during the load.

Device layouts (per core):
  XT  [128d, 8do, 2048t]  x[b]^T via DMA transpose
  QT/KT [128f, 2pair, 2048t] head-pair features on partitions
  V   [128t, 16mt, 4h, 65]  per-token values; col 64 is ones so the attention
                            matmul also emits softmax denominators
  OTn [128hd, 2pair, 2048t] normalized attention output, pair-stacked (odd
                            head shifted to partitions 64:128 via SBUF DMA)

Attention per (pair, q-block of 512), streaming 16 k-tiles:
  S^T(h0), S^T(h1) -> one 2-bank PSUM tile; one exp ACTIVATE over both banks
  (scale=1/8 folded in; logits are O(1) so no max-subtraction is needed)
  -> E [128, 2, 512] fp16; OT_h += [V_h|1].T @ E_h (PSUM accumulation).
The OT accumulators are copied to SBUF immediately (releasing their PSUM
banks) and normalized there: r = 1/OT[64] via reciprocal_approx_fast at
partition 0, gpsimd partition-broadcast, OTn = OT[:64] * r.

The attention stream is ScalarE(exp)-bound, so everything else (V projection,
pair-1 Q/K projections, out-projection) is emitted *between* attention blocks:
Tile's priority scheduler slots that PE work into the exp stream's PE idle.
PSUM: 4 banks S^T (double-buffered 2-bank tiles) + 2 banks OT accumulators +
2 banks for projection/out-projection accumulators.
"""

import sys

for _p in ("/opt/trn_rl_repo", "/opt/pypackages"):
    if _p not in sys.path:
        sys.path.insert(0, _p)

import numpy as np

import concourse.bacc as bacc
import concourse.mybir as mybir
import concourse.tile as tile
from concourse.bass_utils import run_bass_kernel_spmd

F32 = mybir.dt.float32
F16 = mybir.dt.float16
AF = mybir.ActivationFunctionType

B, L, D = 2, 2048, 1024
NH, HD = 16, 64
P = 128
NCORES = 8
NHL = 4                 # heads per core
GH = NHL * HD           # 256: per-core feature-group width
DO = D // P             # 8 contraction subtiles
NT = L // P             # 16 token tiles
NQB = 4                 # q blocks
QW = L // NQB           # 512
SCALE = float(HD) ** -0.5

_CACHE = {}


def _build():
    nc = bacc.Bacc("TRN2", target_bir_lowering=False, debug=False)

    # all inputs arrive pre-packed in device layout (partition-major with
    # multi-KB contiguous per-partition lines -> full DMA bandwidth)
    xt16 = nc.dram_tensor("xt16", [NQB, P, DO, QW], F16, kind="ExternalInput")
    wq = nc.dram_tensor("wq", [P, DO, GH], F16, kind="ExternalInput")
    wk = nc.dram_tensor("wk", [P, DO, GH], F16, kind="ExternalInput")
    wv = nc.dram_tensor("wv", [P, DO, GH], F16, kind="ExternalInput")
    wo = nc.dram_tensor("wo", [P, 2, D], F16, kind="ExternalInput")
    bq = nc.dram_tensor("bq", [P, 2], F32, kind="ExternalInput")
    bk = nc.dram_tensor("bk", [P, 2], F32, kind="ExternalInput")
    bv = nc.dram_tensor("bv", [GH], F32, kind="ExternalInput")
    y = nc.dram_tensor("y", [L, D], F32, kind="ExternalOutput")

    with tile.TileContext(nc) as tc:
        with tc.tile_pool(name="persist", bufs=1) as persist, \
             tc.tile_pool(name="wpool", bufs=1) as wpool, \
             tc.tile_pool(name="xt_pool", bufs=1) as xt_pool, \
             tc.tile_pool(name="epool", bufs=18) as epool, \
             tc.tile_pool(name="npool", bufs=4) as npool, \
             tc.tile_pool(name="ypool", bufs=3) as ypool, \
             tc.tile_pool(name="ps_s", bufs=2, space="PSUM") as ps_s, \
             tc.tile_pool(name="ps_o", bufs=1, space="PSUM") as ps_o, \
             tc.tile_pool(name="ps_p", bufs=1, space="PSUM") as ps_p:

            # ---- PE warmup: ~10us of zero matmuls so HAM un-throttles while
            # the input DMAs stream in.
            wz = persist.tile([P, QW], F16)
            nc.vector.memset(wz[:], 0.0)
            pwm = ps_p.tile([P, QW], F32, tag="pj", name="pwm")
            for _ in range(14):
                nc.tensor.matmul(pwm[:], wz[:, 0:P], wz[:], start=True, stop=True)

            qt = persist.tile([P, 2, L], F16)
            kt = persist.tile([P, 2, L], F16)
            qts = [qt[:, i, :] for i in range(2)]
            kts = [kt[:, i, :] for i in range(2)]
            v = persist.tile([P, NT, NHL, HD + 1], F16)
            otn = persist.tile([P, 2, L], F16)

            # x arrives pre-transposed from the host; stream it in chunks on
            # the sync HWDGE queue (first token block in per-do pieces so the
            # first K-projection group can start ASAP) while the scalar HWDGE
            # queue loads the weights in parallel.
            xts = [xt_pool.tile([P, DO, QW], F16, name=f"xt{i}", tag=f"xt{i}")
                   for i in range(NQB)]
            def xt_chunk(q, tcb):
                q.dma_start(xts[tcb][:], xt16[tcb])

            wk_sb = wpool.tile([P, DO, GH], F16)
            wq_sb = wpool.tile([P, DO, GH], F16)
            wv_sb = wpool.tile([P, DO, GH], F16)
            wo_sb = wpool.tile([P, 2, D], F16)
            bq_sb = persist.tile([P, 2], F32)
            bk_sb = persist.tile([P, 2], F32)
            bv_row = persist.tile([1, GH], F32)
            # NOTE: never issue DMAs on the scalar queue — the exp ACTIVATEs
            # would queue behind them (strict FIFO) and the whole attention
            # stream is exp-paced. Bulk loads go on sync; gpsimd (SWDGE)
            # carries the later chunks in parallel.
            # single queue => in-order service at full HBM bandwidth, in
            # critical-path order (the load is HBM-bound at ~350GB/s).
            nc.sync.dma_start(wk_sb[:], wk[:])
            nc.sync.dma_start(wq_sb[:], wq[:])
            xt_chunk(nc.sync, 0)
            xt_chunk(nc.sync, 1)
            xt_chunk(nc.sync, 2)
            xt_chunk(nc.sync, 3)
            nc.sync.dma_start(wv_sb[:], wv[:])
            nc.sync.dma_start(wo_sb[:], wo[:])
            nc.gpsimd.dma_start(bq_sb[:], bq[:])
            nc.gpsimd.dma_start(bk_sb[:], bk[:])
            nc.gpsimd.dma_start(bv_row[:], bv[None, :])
            bv_sb = persist.tile([P, GH], F32)
            nc.gpsimd.partition_broadcast(bv_sb[:], bv_row[:])
            # ones column for the denominator rows
            ones_sb = persist.tile([P, 1], F32)
            nc.vector.memset(ones_sb[:], 1.0)
            nc.vector.tensor_copy(
                v[:, :, :, HD], ones_sb[:, 0:1, None].to_broadcast((P, NT, NHL)))

            def proj_v(mt):
                pv = ps_p.tile([P, QW], F32, tag="pj", name="pv")
                for do in range(DO):
                    nc.tensor.matmul(
                        pv[:, 0:GH],
                        xts[mt // 4][:, do, (mt % 4) * P:(mt % 4 + 1) * P],
                        wv_sb[:, do, :],
                        start=(do == 0), stop=(do == DO - 1),
                    )
                nc.vector.tensor_add(
                    out=v[:, mt, :, 0:HD],
                    in0=pv[:, 0:GH].rearrange("p (h c) -> p h c", h=NHL),
                    in1=bv_sb[:].rearrange("p (h c) -> p h c", h=NHL),
                )

            def proj_qk(w_sb, b_sb, dst, pr, tcb):
                pj = ps_p.tile([P, QW], F32, tag="pj", name="pj")
                for do in range(DO):
                    nc.tensor.matmul(
                        pj[:],
                        w_sb[:, do, pr * P:(pr + 1) * P],
                        xts[tcb][:, do, :],
                        start=(do == 0), stop=(do == DO - 1),
                    )
                nc.vector.tensor_scalar_add(
                    dst[pr][:, tcb * QW:(tcb + 1) * QW], pj[:], b_sb[:, pr:pr + 1])

            def proj_qk_gen(w_sb, b_sb, dst, pr):
                # generator form: 2 matmuls per step, one step per attention
                # k-tile, so the projection never delays the next S^T pair.
                for tcb in range(NQB):
                    pj = ps_p.tile([P, QW], F32, tag="pj", name="pj")
                    for do in range(DO):
                        nc.tensor.matmul(
                            pj[:],
                            w_sb[:, do, pr * P:(pr + 1) * P],
                            xts[tcb][:, do, :],
                            start=(do == 0), stop=(do == DO - 1),
                        )
                        if do % 2 == 1:
                            yield
                    nc.vector.tensor_scalar_add(
                        dst[pr][:, tcb * QW:(tcb + 1) * QW], pj[:], b_sb[:, pr:pr + 1])

            def outproj_gen(mts):
                for mt in mts:
                    y_sb = ypool.tile([P, D], F32, name="y_sb")
                    for nh in range(2):
                        py = ps_p.tile([P, QW], F32, tag="pj", name="py")
                        for pr in range(2):
                            nc.tensor.matmul(
                                py[:],
                                otn[:, pr, mt * P:(mt + 1) * P],
                                wo_sb[:, pr, nh * QW:(nh + 1) * QW],
                                start=(pr == 0), stop=(pr == 1),
                            )
                        nc.vector.tensor_copy(y_sb[:, nh * QW:(nh + 1) * QW], py[:])
                        yield
                    nc.sync.dma_start(y[mt * P:(mt + 1) * P, :], y_sb[:])

            _blk = [0]

            def attention(pr, qb, fuse_vproj=False, filler=None, filler_start=0):
                qs = slice(qb * QW, (qb + 1) * QW)
                # rotate accumulator banks (3 tags, pairs) so a new block's
                # OT matmuls never wait on the previous block's evacuation.
                bi = _blk[0]; _blk[0] += 1
                po = [ps_o.tile([HD + 1, QW], F32, tag=f"po{(2 * bi + i) % 3}",
                                name=f"po{i}")
                      for i in range(2)]
                for ktile in range(NT):
                    pS = ps_s.tile([P, 2, QW], F32, name="pS")
                    for hi in range(2):
                        lo = hi * HD
                        nc.tensor.matmul(
                            pS[:, hi, :],
                            kts[pr][lo:lo + HD, ktile * P:(ktile + 1) * P],
                            qts[pr][lo:lo + HD, qs],
                            start=True, stop=True,
                        )
                    if fuse_vproj:
                        # v[ktile] is first read by this iteration's OT matmul;
                        # emitting its projection here (below the S^T matmuls in
                        # priority) lets the exp stream start ~30us earlier.
                        proj_v(ktile)
                    e_sb = epool.tile([P, 2, QW], F16, name="e_sb")
                    nc.scalar.activation(e_sb[:], pS[:], AF.Exp, scale=SCALE)
                    # filler sits between exp and the OT pair: the in-order PE
                    # stream is blocked on exp here anyway, so this is free PE
                    # time. filler_start>0 skips early k-tiles when the first
                    # piece depends on the previous block's normalize chain.
                    if filler is not None and ktile >= filler_start:
                        next(filler, None)
                    for hi in range(2):
                        nc.tensor.matmul(
                            po[hi][:],
                            v[:, ktile, 2 * pr + hi, :],
                            e_sb[:, hi, :],
                            start=(ktile == 0), stop=(ktile == NT - 1),
                        )
                if filler is not None:
                    for _ in filler:
                        pass
                for hi in range(2):
                    # Evacuate the accumulator to SBUF right away (frees its
                    # PSUM bank for the next block); normalize from the copy.
                    ou = npool.tile([HD + 1, QW], F32, tag="ou", name="ou")
                    nc.vector.tensor_copy(ou[:], po[hi][:])
                    s0 = npool.tile([1, QW], F32, tag="s0", name="s0")
                    nc.sync.dma_start(s0[:], ou[HD:HD + 1, :])
                    r0 = npool.tile([1, QW], F32, tag="r0", name="r0")
                    nc.vector.reciprocal_approx_fast(r0[:], s0[:])
                    rb = npool.tile([HD, QW], F32, tag="rb", name="rb")
                    nc.gpsimd.partition_broadcast(rb[:], r0[:])
                    if hi == 0:
                        nc.vector.tensor_mul(
                            out=otn[0:HD, pr, qs], in0=ou[0:HD, :], in1=rb[:])
                    else:
                        tmp = npool.tile([HD, QW], F16, tag="tmp", name="tmp")
                        nc.vector.tensor_mul(out=tmp[:], in0=ou[0:HD, :], in1=rb[:])
                        nc.sync.dma_start(otn[HD:P, pr, qs], tmp[:])

            def outproj(mt, ps_y):
                y_sb = ypool.tile([P, D], F32, name="y_sb")
                for nh in range(2):
                    py = ps_y.tile([P, QW], F32, name="py")
                    for pr in range(2):
                        nc.tensor.matmul(
                            py[:],
                            otn[:, pr, mt * P:(mt + 1) * P],
                            wo_sb[:, pr, nh * QW:(nh + 1) * QW],
                            start=(pr == 0), stop=(pr == 1),
                        )
                    nc.vector.tensor_copy(y_sb[:, nh * QW:(nh + 1) * QW], py[:])
                nc.sync.dma_start(y[mt * P:(mt + 1) * P, :], y_sb[:])

            # ---- schedule: pair-0 K/Q first so the exp stream starts early
            # (S^T needs only K/Q; the deep E pool lets exp run ahead of the
            # V-dependent OT matmuls while the PE catches up on V). Pair-1
            # projections are emitted between pair-0 attention blocks to fill
            # the exp stream's PE idle. NOTE: Tile dependencies follow
            # emission order, so every tensor must be emitted-written before
            # an emitted-read (no read-before-write reordering).
            for tcb in range(NQB):
                proj_qk(wk_sb, bk_sb, kts, 0, tcb)
                proj_qk(wq_sb, bq_sb, qts, 0, tcb)
            attention(0, 0, fuse_vproj=True)
            attention(0, 1, filler=proj_qk_gen(wk_sb, bk_sb, kts, 1))
            attention(0, 2, filler=proj_qk_gen(wq_sb, bq_sb, qts, 1))
            attention(0, 3)
            # out-projection for q-block qb fills the next attention
            # block's exp-paced PE idle.
            attention(1, 0)
            attention(1, 1, filler=outproj_gen(range(0, 4)), filler_start=4)
            attention(1, 2, filler=outproj_gen(range(4, 8)), filler_start=4)
            attention(1, 3, filler=outproj_gen(range(8, 12)), filler_start=4)
            for _ in outproj_gen(range(12, NT)):
                pass

    nc.compile()
    return nc


def _get_nc():
    if "nc" not in _CACHE:
        _CACHE["nc"] = _build()
    return _CACHE["nc"]


def kernel(x, W_qkv, b_qkv, W_out, b_out):
    x = np.asarray(x, dtype=np.float32)
    W_qkv16 = np.asarray(W_qkv, dtype=np.float32).astype(np.float16)
    b_qkv = np.asarray(b_qkv, dtype=np.float32)
    W_out16 = np.asarray(W_out, dtype=np.float32).astype(np.float16)
    b_out = np.asarray(b_out, dtype=np.float32)

    nc = _get_nc()

    def pack_w(w2d):  # [K, C] -> [P, K//P, C]
        return np.ascontiguousarray(
            w2d.reshape(w2d.shape[0] // P, P, -1).transpose(1, 0, 2))

    xt16s = [np.ascontiguousarray(
        x[b].astype(np.float16).reshape(NQB, QW, DO, P).transpose(0, 3, 2, 1))
        for b in range(B)]
    in_maps = []
    for core in range(NCORES):
        b = core // 4
        c0 = (core % 4) * GH
        in_maps.append({
            "xt16": xt16s[b],
            "wq": pack_w(W_qkv16[:, c0:c0 + GH]),
            "wk": pack_w(W_qkv16[:, D + c0:D + c0 + GH]),
            "wv": pack_w(W_qkv16[:, 2 * D + c0:2 * D + c0 + GH]),
            "wo": pack_w(W_out16[c0:c0 + GH, :]),
            "bq": np.ascontiguousarray(b_qkv[c0:c0 + GH].reshape(2, P).T),
            "bk": np.ascontiguousarray(b_qkv[D + c0:D + c0 + GH].reshape(2, P).T),
            "bv": np.ascontiguousarray(b_qkv[2 * D + c0:2 * D + c0 + GH]),
        })

    import os
    trace = bool(int(os.environ.get("BASS_KERNEL_TRACE", "0")))
    res = run_bass_kernel_spmd(nc, in_maps, list(range(NCORES)), trace=trace)
    _CACHE["last_result"] = res

    parts = [res.results[i]["y"] for i in range(NCORES)]
    out = np.empty((B, L, D), dtype=np.float32)
    out[0] = parts[0] + parts[1] + parts[2] + parts[3]
    out[1] = parts[4] + parts[5] + parts[6] + parts[7]
    out += b_out
    return out


# revision 48
# speedup vs baseline: 1.0687x; 1.0687x over previous
"""Multi-head self-attention (B=2, L=2048, D=1024, H=16) on 8 Trainium2 cores.

Sharding: batch x head-group. Core c handles batch c//4 and heads
(c%4)*4 .. (c%4)*4+4. Each core computes Q/K/V projections for its 4 heads,
attention, and a partial out-projection (row-slice of W_out); the host sums
the 4 partials per batch and adds b_out.

The device pipeline runs in fp16 (fp32 PSUM accumulation everywhere): fp16
matmuls stream at 1 cycle/row on the PE vs 2 for fp32r, and 2-byte dtypes get
the XBAR DMA-transpose path so x is transposed during the load.

Device layouts (per core):
  XT  [128d, 8do, 2048t]  x[b]^T via DMA transpose
  QT/KT [128f, 2pair, 2048t] head-pair features on partitions
  V   [128t, 16mt, 4h, 65]  per-token values; col 64 is ones so the attention
                            matmul also emits softmax denominators
  OTn [128hd, 2pair, 2048t] normalized attention output, pair-stacked (odd
                            head shifted to partitions 64:128 via SBUF DMA)

Attention per (pair, q-block of 512), streaming 16 k-tiles:
  S^T(h0), S^T(h1) -> one 2-bank PSUM tile; one exp ACTIVATE over both banks
  (scale=1/8 folded in; logits are O(1) so no max-subtraction is needed)
  -> E [128, 2, 512] fp16; OT_h += [V_h|1].T @ E_h (PSUM accumulation).
The OT accumulators are copied to SBUF immediately (releasing their PSUM
banks) and normalized there: r = 1/OT[64] via reciprocal_approx_fast at
partition 0, gpsimd partition-broadcast, OTn = OT[:64] * r.

The attention stream is ScalarE(exp)-bound, so everything else (V projection,
pair-1 Q/K projections, out-projection) is emitted *between* attention blocks:
Tile's priority scheduler slots that PE work into the exp stream's PE idle.
PSUM: 4 banks S^T (double-buffered 2-bank tiles) + 2 banks OT accumulators +
2 banks for projection/out-projection accumulators.
"""

import sys

for _p in ("/opt/trn_rl_repo", "/opt/pypackages"):
    if _p not in sys.path:
        sys.path.insert(0, _p)

import numpy as np

import concourse.bacc as bacc
import concourse.mybir as mybir
import concourse.tile as tile
from concourse.bass_utils import run_bass_kernel_spmd

F32 = mybir.dt.float32
F16 = mybir.dt.float16
AF = mybir.ActivationFunctionType

B, L, D = 2, 2048, 1024
NH, HD = 16, 64
P = 128
NCORES = 8
NHL = 4                 # heads per core
GH = NHL * HD           # 256: per-core feature-group width
DO = D // P             # 8 contraction subtiles
NT = L // P             # 16 token tiles
NQB = 4                 # q blocks
QW = L // NQB           # 512
SCALE = float(HD) ** -0.5

_CACHE = {}


def _build():
    nc = bacc.Bacc("TRN2", target_bir_lowering=False, debug=False)

    # all inputs arrive pre-packed in device layout (partition-major with
    # multi-KB contiguous per-partition lines -> full DMA bandwidth)
    xt16 = nc.dram_tensor("xt16", [NQB, P, DO, QW], F16, kind="ExternalInput")
    wq = nc.dram_tensor("wq", [P, DO, GH], F16, kind="ExternalInput")
    wk = nc.dram_tensor("wk", [P, DO, GH], F16, kind="ExternalInput")
    wv = nc.dram_tensor("wv", [P, DO, GH], F16, kind="ExternalInput")
    wo = nc.dram_tensor("wo", [P, 2, D], F16, kind="ExternalInput")
    bq = nc.dram_tensor("bq", [P, 2], F32, kind="ExternalInput")
    bk = nc.dram_tensor("bk", [P, 2], F32, kind="ExternalInput")
    bv = nc.dram_tensor("bv", [GH], F32, kind="ExternalInput")
    y = nc.dram_tensor("y", [L, D], F32, kind="ExternalOutput")

    with tile.TileContext(nc) as tc:
        with tc.tile_pool(name="persist", bufs=1) as persist, \
             tc.tile_pool(name="wpool", bufs=1) as wpool, \
             tc.tile_pool(name="xt_pool", bufs=1) as xt_pool, \
             tc.tile_pool(name="epool", bufs=18) as epool, \
             tc.tile_pool(name="npool", bufs=4) as npool, \
             tc.tile_pool(name="ypool", bufs=3) as ypool, \
             tc.tile_pool(name="ps_s", bufs=2, space="PSUM") as ps_s, \
             tc.tile_pool(name="ps_o", bufs=1, space="PSUM") as ps_o:
            from contextlib import ExitStack
            _psP = ExitStack()
            ps_p = _psP.enter_context(tc.tile_pool(name="ps_p", bufs=2, space="PSUM"))

            # ---- PE warmup: ~10us of zero matmuls so HAM un-throttles while
            # the input DMAs stream in.
            wz = persist.tile([P, QW], F16)
            nc.vector.memset(wz[:], 0.0)
            pwm = ps_p.tile([P, QW], F32, tag="pj", name="pwm")
            for _ in range(14):
                nc.tensor.matmul(pwm[:], wz[:, 0:P], wz[:], start=True, stop=True)

            qt = persist.tile([P, 2, L], F16)
            kt = persist.tile([P, 2, L], F16)
            qts = [qt[:, i, :] for i in range(2)]
            kts = [kt[:, i, :] for i in range(2)]
            v = persist.tile([P, NT, NHL, HD + 1], F16)
            otn = persist.tile([P, 2, L], F16)

            # x arrives pre-transposed from the host; stream it in chunks on
            # the sync HWDGE queue (first token block in per-do pieces so the
            # first K-projection group can start ASAP) while the scalar HWDGE
            # queue loads the weights in parallel.
            xts = [xt_pool.tile([P, DO, QW], F16, name=f"xt{i}", tag=f"xt{i}")
                   for i in range(NQB)]
            def xt_chunk(q, tcb):
                q.dma_start(xts[tcb][:], xt16[tcb])

            wk_sb = wpool.tile([P, DO, GH], F16)
            wq_sb = wpool.tile([P, DO, GH], F16)
            wv_sb = wpool.tile([P, DO, GH], F16)
            wo_sb = wpool.tile([P, 2, D], F16)
            bq_sb = persist.tile([P, 2], F32)
            bk_sb = persist.tile([P, 2], F32)
            bv_row = persist.tile([1, GH], F32)
            # NOTE: never issue DMAs on the scalar queue — the exp ACTIVATEs
            # would queue behind them (strict FIFO) and the whole attention
            # stream is exp-paced. Bulk loads go on sync; gpsimd (SWDGE)
            # carries the later chunks in parallel.
            # single queue => in-order service at full HBM bandwidth, in
            # critical-path order (the load is HBM-bound at ~350GB/s).
            nc.sync.dma_start(wk_sb[:], wk[:])
            nc.sync.dma_start(wq_sb[:], wq[:])
            xt_chunk(nc.sync, 0)
            xt_chunk(nc.sync, 1)
            xt_chunk(nc.sync, 2)
            xt_chunk(nc.sync, 3)
            nc.sync.dma_start(wv_sb[:], wv[:])
            nc.sync.dma_start(wo_sb[:], wo[:])
            nc.gpsimd.dma_start(bq_sb[:], bq[:])
            nc.gpsimd.dma_start(bk_sb[:], bk[:])
            nc.gpsimd.dma_start(bv_row[:], bv[None, :])
            bv_sb = persist.tile([P, GH], F32)
            nc.gpsimd.partition_broadcast(bv_sb[:], bv_row[:])
            # ones column for the denominator rows
            ones_sb = persist.tile([P, 1], F32)
            nc.vector.memset(ones_sb[:], 1.0)
            nc.vector.tensor_copy(
                v[:, :, :, HD], ones_sb[:, 0:1, None].to_broadcast((P, NT, NHL)))

            def proj_v(mt):
                pv = ps_p.tile([P, QW], F32, tag="pj", name="pv")
                for do in range(DO):
                    nc.tensor.matmul(
                        pv[:, 0:GH],
                        xts[mt // 4][:, do, (mt % 4) * P:(mt % 4 + 1) * P],
                        wv_sb[:, do, :],
                        start=(do == 0), stop=(do == DO - 1),
                    )
                nc.vector.tensor_add(
                    out=v[:, mt, :, 0:HD],
                    in0=pv[:, 0:GH].rearrange("p (h c) -> p h c", h=NHL),
                    in1=bv_sb[:].rearrange("p (h c) -> p h c", h=NHL),
                )

            def proj_qk(w_sb, b_sb, dst, pr, tcb):
                pj = ps_p.tile([P, QW], F32, tag="pj", name="pj")
                for do in range(DO):
                    nc.tensor.matmul(
                        pj[:],
                        w_sb[:, do, pr * P:(pr + 1) * P],
                        xts[tcb][:, do, :],
                        start=(do == 0), stop=(do == DO - 1),
                    )
                nc.vector.tensor_scalar_add(
                    dst[pr][:, tcb * QW:(tcb + 1) * QW], pj[:], b_sb[:, pr:pr + 1])

            def proj_qk_gen(w_sb, b_sb, dst, pr):
                # generator form: 2 matmuls per step, one step per attention
                # k-tile, so the projection never delays the next S^T pair.
                for tcb in range(NQB):
                    pj = ps_p.tile([P, QW], F32, tag="pj", name="pj")
                    for do in range(DO):
                        nc.tensor.matmul(
                            pj[:],
                            w_sb[:, do, pr * P:(pr + 1) * P],
                            xts[tcb][:, do, :],
                            start=(do == 0), stop=(do == DO - 1),
                        )
                        if do % 2 == 1:
                            yield
                    nc.vector.tensor_scalar_add(
                        dst[pr][:, tcb * QW:(tcb + 1) * QW], pj[:], b_sb[:, pr:pr + 1])

            def outproj_gen(mts, ps_y):
                for mt in mts:
                    y_sb = ypool.tile([P, D], F32, name="y_sb")
                    for nh in range(2):
                        py = ps_y.tile([P, QW], F32, name="py")
                        for pr in range(2):
                            nc.tensor.matmul(
                                py[:],
                                otn[:, pr, mt * P:(mt + 1) * P],
                                wo_sb[:, pr, nh * QW:(nh + 1) * QW],
                                start=(pr == 0), stop=(pr == 1),
                            )
                        nc.vector.tensor_copy(y_sb[:, nh * QW:(nh + 1) * QW], py[:])
                        yield
                    nc.sync.dma_start(y[mt * P:(mt + 1) * P, :], y_sb[:])

            def attention(pr, qb, fuse_vproj=False, filler=None, filler_start=0):
                qs = slice(qb * QW, (qb + 1) * QW)
                po = [ps_o.tile([HD + 1, QW], F32, tag=f"po{i}", name=f"po{i}")
                      for i in range(2)]
                OT_LAG = 2
                es = {}

                def emit_ot(kt):
                    for hi in range(2):
                        nc.tensor.matmul(
                            po[hi][:],
                            v[:, kt, 2 * pr + hi, :],
                            es[kt][:, hi, :],
                            start=(kt == 0), stop=(kt == NT - 1),
                        )

                for ktile in range(NT):
                    pS = ps_s.tile([P, 2, QW], F32, name="pS")
                    for hi in range(2):
                        lo = hi * HD
                        nc.tensor.matmul(
                            pS[:, hi, :],
                            kts[pr][lo:lo + HD, ktile * P:(ktile + 1) * P],
                            qts[pr][lo:lo + HD, qs],
                            start=True, stop=True,
                        )
                    if fuse_vproj:
                        # v[ktile] is first read by this block's OT matmuls;
                        # emitting its projection here (below the S^T matmuls
                        # in priority) lets the exp stream start ~30us earlier.
                        proj_v(ktile)
                    es[ktile] = epool.tile([P, 2, QW], F16, name="e_sb")
                    nc.scalar.activation(es[ktile][:], pS[:], AF.Exp, scale=SCALE)
                    # filler sits between exp and the (lagged) OT pair: the
                    # in-order PE stream is waiting on exp here, so this is
                    # free PE time. filler_start>0 skips early k-tiles when the
                    # first piece depends on the previous block's normalize.
                    if filler is not None and ktile >= filler_start:
                        next(filler, None)
                    # OTs lag 2 k-tiles: a new block's first STs (which feed the
                    # exp stream) are never queued behind the OT matmuls that
                    # wait on the previous block's accumulator evacuation.
                    if ktile >= OT_LAG:
                        emit_ot(ktile - OT_LAG)
                for kt in range(NT - OT_LAG, NT):
                    emit_ot(kt)
                if filler is not None:
                    for _ in filler:
                        pass
                for hi in range(2):
                    # Evacuate the accumulator to SBUF right away (frees its
                    # PSUM bank for the next block); normalize from the copy.
                    ou = npool.tile([HD + 1, QW], F32, tag="ou", name="ou")
                    nc.vector.tensor_copy(ou[:], po[hi][:])
                    s0 = npool.tile([1, QW], F32, tag="s0", name="s0")
                    nc.sync.dma_start(s0[:], ou[HD:HD + 1, :])
                    r0 = npool.tile([1, QW], F32, tag="r0", name="r0")
                    nc.vector.reciprocal_approx_fast(r0[:], s0[:])
                    rb = npool.tile([HD, QW], F32, tag="rb", name="rb")
                    nc.gpsimd.partition_broadcast(rb[:], r0[:])
                    if hi == 0:
                        nc.vector.tensor_mul(
                            out=otn[0:HD, pr, qs], in0=ou[0:HD, :], in1=rb[:])
                    else:
                        tmp = npool.tile([HD, QW], F16, tag="tmp", name="tmp")
                        nc.vector.tensor_mul(out=tmp[:], in0=ou[0:HD, :], in1=rb[:])
                        nc.sync.dma_start(otn[HD:P, pr, qs], tmp[:])

            def outproj(mt, ps_y):
                y_sb = ypool.tile([P, D], F32, name="y_sb")
                for nh in range(2):
                    py = ps_y.tile([P, QW], F32, name="py")
                    for pr in range(2):
                        nc.tensor.matmul(
                            py[:],
                            otn[:, pr, mt * P:(mt + 1) * P],
                            wo_sb[:, pr, nh * QW:(nh + 1) * QW],
                            start=(pr == 0), stop=(pr == 1),
                        )
                    nc.vector.tensor_copy(y_sb[:, nh * QW:(nh + 1) * QW], py[:])
                nc.sync.dma_start(y[mt * P:(mt + 1) * P, :], y_sb[:])

            # ---- schedule: pair-0 K/Q first so the exp stream starts early
            # (S^T needs only K/Q; the deep E pool lets exp run ahead of the
            # V-dependent OT matmuls while the PE catches up on V). Pair-1
            # projections are emitted between pair-0 attention blocks to fill
            # the exp stream's PE idle. NOTE: Tile dependencies follow
            # emission order, so every tensor must be emitted-written before
            # an emitted-read (no read-before-write reordering).
            for tcb in range(NQB):
                proj_qk(wk_sb, bk_sb, kts, 0, tcb)
                proj_qk(wq_sb, bq_sb, qts, 0, tcb)
            attention(0, 0, fuse_vproj=True)
            attention(0, 1, filler=proj_qk_gen(wk_sb, bk_sb, kts, 1))
            attention(0, 2, filler=proj_qk_gen(wq_sb, bq_sb, qts, 1))
            attention(0, 3)
            _psP.close()
            with tc.tile_pool(name="ps_y", bufs=2, space="PSUM") as ps_y:
                # out-projection for q-block qb fills the next attention
                # block's exp-paced PE idle.
                attention(1, 0)
                attention(1, 1, filler=outproj_gen(range(0, 4), ps_y), filler_start=4)
                attention(1, 2, filler=outproj_gen(range(4, 8), ps_y), filler_start=4)
                attention(1, 3, filler=outproj_gen(range(8, 12), ps_y), filler_start=4)
                for _ in outproj_gen(range(12, NT), ps_y):
                    pass

    nc.compile()
    return nc


def _get_nc():
    if "nc" not in _CACHE:
        _CACHE["nc"] = _build()
    return _CACHE["nc"]


def kernel(x, W_qkv, b_qkv, W_out, b_out):
    x = np.asarray(x, dtype=np.float32)
    W_qkv16 = np.asarray(W_qkv, dtype=np.float32).astype(np.float16)
    b_qkv = np.asarray(b_qkv, dtype=np.float32)
    W_out16 = np.asarray(W_out, dtype=np.float32).astype(np.float16)
    b_out = np.asarray(b_out, dtype=np.float32)

    nc = _get_nc()

    def pack_w(w2d):  # [K, C] -> [P, K//P, C]
        return np.ascontiguousarray(
            w2d.reshape(w2d.shape[0] // P, P, -1).transpose(1, 0, 2))

    xt16s = [np.ascontiguousarray(
        x[b].astype(np.float16).reshape(NQB, QW, DO, P).transpose(0, 3, 2, 1))
        for b in range(B)]
    in_maps = []
    for core in range(NCORES):
        b = core // 4
        c0 = (core % 4) * GH
        in_maps.append({
            "xt16": xt16s[b],
            "wq": pack_w(W_qkv16[:, c0:c0 + GH]),
            "wk": pack_w(W_qkv16[:, D + c0:D + c0 + GH]),
            "wv": pack_w(W_qkv16[:, 2 * D + c0:2 * D + c0 + GH]),
            "wo": pack_w(W_out16[c0:c0 + GH, :]),
            "bq": np.ascontiguousarray(b_qkv[c0:c0 + GH].reshape(2, P).T),
            "bk": np.ascontiguousarray(b_qkv[D + c0:D + c0 + GH].reshape(2, P).T),
            "bv": np.ascontiguousarray(b_qkv[2 * D + c0:2 * D + c0 + GH]),
        })

    import os
    trace = bool(int(os.environ.get("BASS_KERNEL_TRACE", "0")))
    res = run_bass_kernel_spmd(nc, in_maps, list(range(NCORES)), trace=trace)
    _CACHE["last_result"] = res

    parts = [res.results[i]["y"] for i in range(NCORES)]
    out = np.empty((B, L, D), dtype=np.float32)
    out[0] = parts[0] + parts[1] + parts[2] + parts[3]
    out[1] = parts[4] + parts[5] + parts[6] + parts[7]
    out += b_out
    return out


# revision 49
# speedup vs baseline: 1.0751x; 1.0061x over previous
"""Multi-head self-attention (B=2, L=2048, D=1024, H=16) on 8 Trainium2 cores.

Sharding: batch x head-group. Core c handles batch c//4 and heads
(c%4)*4 .. (c%4)*4+4. Each core computes Q/K/V projections for its 4 heads,
attention, and a partial out-projection (row-slice of W_out); the host sums
the 4 partials per batch and adds b_out.

The device pipeline runs in fp16 (fp32 PSUM accumulation everywhere): fp16
matmuls stream at 1 cycle/row on the PE vs 2 for fp32r, and 2-byte dtypes get
the XBAR DMA-transpose path so x is transposed during the load.

Device layouts (per core):
  XT  [128d, 8do, 2048t]  x[b]^T via DMA transpose
  QT/KT [128f, 2pair, 2048t] head-pair features on partitions
  V   [128t, 16mt, 4h, 65]  per-token values; col 64 is ones so the attention
                            matmul also emits softmax denominators
  OTn [128hd, 2pair, 2048t] normalized attention output, pair-stacked (odd
                            head shifted to partitions 64:128 via SBUF DMA)

Attention per (pair, q-block of 512), streaming 16 k-tiles:
  S^T(h0), S^T(h1) -> one 2-bank PSUM tile; one exp ACTIVATE over both banks
  (scale=1/8 folded in; logits are O(1) so no max-subtraction is needed)
  -> E [128, 2, 512] fp16; OT_h += [V_h|1].T @ E_h (PSUM accumulation).
The OT accumulators are copied to SBUF immediately (releasing their PSUM
banks) and normalized there: r = 1/OT[64] via reciprocal_approx_fast at
partition 0, gpsimd partition-broadcast, OTn = OT[:64] * r.

The attention stream is ScalarE(exp)-bound, so everything else (V projection,
pair-1 Q/K projections, out-projection) is emitted *between* attention blocks:
Tile's priority scheduler slots that PE work into the exp stream's PE idle.
PSUM: 4 banks S^T (double-buffered 2-bank tiles) + 2 banks OT accumulators +
2 banks for projection/out-projection accumulators.
"""

import sys

for _p in ("/opt/trn_rl_repo", "/opt/pypackages"):
    if _p not in sys.path:
        sys.path.insert(0, _p)

import numpy as np

import concourse.bacc as bacc
import concourse.mybir as mybir
import concourse.tile as tile
from concourse.bass_utils import run_bass_kernel_spmd

F32 = mybir.dt.float32
F16 = mybir.dt.float16
AF = mybir.ActivationFunctionType

B, L, D = 2, 2048, 1024
NH, HD = 16, 64
P = 128
NCORES = 8
NHL = 4                 # heads per core
GH = NHL * HD           # 256: per-core feature-group width
DO = D // P             # 8 contraction subtiles
NT = L // P             # 16 token tiles
NQB = 4                 # q blocks
QW = L // NQB           # 512
SCALE = float(HD) ** -0.5

_CACHE = {}


def _build():
    nc = bacc.Bacc("TRN2", target_bir_lowering=False, debug=False)

    # all inputs arrive pre-packed in device layout (partition-major with
    # multi-KB contiguous per-partition lines -> full DMA bandwidth)
    xt16 = nc.dram_tensor("xt16", [NQB, P, DO, QW], F16, kind="ExternalInput")
    wq = nc.dram_tensor("wq", [P, DO, GH], F16, kind="ExternalInput")
    wk = nc.dram_tensor("wk", [P, DO, GH], F16, kind="ExternalInput")
    wv = nc.dram_tensor("wv", [P, DO, GH], F16, kind="ExternalInput")
    wo = nc.dram_tensor("wo", [P, 2, D], F16, kind="ExternalInput")
    bq = nc.dram_tensor("bq", [P, 2], F32, kind="ExternalInput")
    bk = nc.dram_tensor("bk", [P, 2], F32, kind="ExternalInput")
    bv = nc.dram_tensor("bv", [GH], F32, kind="ExternalInput")
    y = nc.dram_tensor("y", [L, D], F32, kind="ExternalOutput")

    with tile.TileContext(nc) as tc:
        with tc.tile_pool(name="persist", bufs=1) as persist, \
             tc.tile_pool(name="wpool", bufs=1) as wpool, \
             tc.tile_pool(name="xt_pool", bufs=1) as xt_pool, \
             tc.tile_pool(name="epool", bufs=18) as epool, \
             tc.tile_pool(name="npool", bufs=4) as npool, \
             tc.tile_pool(name="ypool", bufs=3) as ypool, \
             tc.tile_pool(name="ps_s", bufs=2, space="PSUM") as ps_s, \
             tc.tile_pool(name="ps_o", bufs=1, space="PSUM") as ps_o:
            from contextlib import ExitStack
            _psP = ExitStack()
            ps_p = _psP.enter_context(tc.tile_pool(name="ps_p", bufs=2, space="PSUM"))

            # ---- PE warmup: ~10us of zero matmuls so HAM un-throttles while
            # the input DMAs stream in.
            wz = persist.tile([P, QW], F16)
            nc.vector.memset(wz[:], 0.0)
            pwm = ps_p.tile([P, QW], F32, tag="pj", name="pwm")
            for _ in range(14):
                nc.tensor.matmul(pwm[:], wz[:, 0:P], wz[:], start=True, stop=True)

            qt = persist.tile([P, 2, L], F16)
            kt = persist.tile([P, 2, L], F16)
            qts = [qt[:, i, :] for i in range(2)]
            kts = [kt[:, i, :] for i in range(2)]
            v = persist.tile([P, NT, NHL, HD + 1], F16)
            otn = persist.tile([P, 2, L], F16)

            # x arrives pre-transposed from the host; stream it in chunks on
            # the sync HWDGE queue (first token block in per-do pieces so the
            # first K-projection group can start ASAP) while the scalar HWDGE
            # queue loads the weights in parallel.
            xts = [xt_pool.tile([P, DO, QW], F16, name=f"xt{i}", tag=f"xt{i}")
                   for i in range(NQB)]
            def xt_chunk(q, tcb):
                q.dma_start(xts[tcb][:], xt16[tcb])

            wk_sb = wpool.tile([P, DO, GH], F16)
            wq_sb = wpool.tile([P, DO, GH], F16)
            wv_sb = wpool.tile([P, DO, GH], F16)
            wo_sb = wpool.tile([P, 2, D], F16)
            bq_sb = persist.tile([P, 2], F32)
            bk_sb = persist.tile([P, 2], F32)
            bv_row = persist.tile([1, GH], F32)
            # NOTE: never issue DMAs on the scalar queue — the exp ACTIVATEs
            # would queue behind them (strict FIFO) and the whole attention
            # stream is exp-paced. Bulk loads go on sync; gpsimd (SWDGE)
            # carries the later chunks in parallel.
            # single queue => in-order service at full HBM bandwidth, in
            # critical-path order (the load is HBM-bound at ~350GB/s).
            nc.sync.dma_start(wk_sb[:], wk[:])
            nc.sync.dma_start(wq_sb[:], wq[:])
            xt_chunk(nc.sync, 0)
            xt_chunk(nc.sync, 1)
            xt_chunk(nc.sync, 2)
            xt_chunk(nc.sync, 3)
            nc.sync.dma_start(wv_sb[:], wv[:])
            nc.sync.dma_start(wo_sb[:], wo[:])
            nc.gpsimd.dma_start(bq_sb[:], bq[:])
            nc.gpsimd.dma_start(bk_sb[:], bk[:])
            nc.gpsimd.dma_start(bv_row[:], bv[None, :])
            bv_sb = persist.tile([P, GH], F32)
            nc.gpsimd.partition_broadcast(bv_sb[:], bv_row[:])
            # ones column for the denominator rows
            ones_sb = persist.tile([P, 1], F32)
            nc.vector.memset(ones_sb[:], 1.0)
            nc.vector.tensor_copy(
                v[:, :, :, HD], ones_sb[:, 0:1, None].to_broadcast((P, NT, NHL)))

            def proj_v(mt):
                pv = ps_p.tile([P, QW], F32, tag="pj", name="pv")
                for do in range(DO):
                    nc.tensor.matmul(
                        pv[:, 0:GH],
                        xts[mt // 4][:, do, (mt % 4) * P:(mt % 4 + 1) * P],
                        wv_sb[:, do, :],
                        start=(do == 0), stop=(do == DO - 1),
                    )
                nc.vector.tensor_add(
                    out=v[:, mt, :, 0:HD],
                    in0=pv[:, 0:GH].rearrange("p (h c) -> p h c", h=NHL),
                    in1=bv_sb[:].rearrange("p (h c) -> p h c", h=NHL),
                )

            def proj_qk(w_sb, b_sb, dst, pr, tcb):
                pj = ps_p.tile([P, QW], F32, tag="pj", name="pj")
                for do in range(DO):
                    nc.tensor.matmul(
                        pj[:],
                        w_sb[:, do, pr * P:(pr + 1) * P],
                        xts[tcb][:, do, :],
                        start=(do == 0), stop=(do == DO - 1),
                    )
                nc.vector.tensor_scalar_add(
                    dst[pr][:, tcb * QW:(tcb + 1) * QW], pj[:], b_sb[:, pr:pr + 1])

            def proj_qk_gen(w_sb, b_sb, dst, pr):
                # generator form: 2 matmuls per step, one step per attention
                # k-tile, so the projection never delays the next S^T pair.
                for tcb in range(NQB):
                    pj = ps_p.tile([P, QW], F32, tag="pj", name="pj")
                    for do in range(DO):
                        nc.tensor.matmul(
                            pj[:],
                            w_sb[:, do, pr * P:(pr + 1) * P],
                            xts[tcb][:, do, :],
                            start=(do == 0), stop=(do == DO - 1),
                        )
                        if do % 2 == 1:
                            yield
                    nc.vector.tensor_scalar_add(
                        dst[pr][:, tcb * QW:(tcb + 1) * QW], pj[:], b_sb[:, pr:pr + 1])

            def outproj_gen(mts, ps_y):
                for mt in mts:
                    y_sb = ypool.tile([P, D], F32, name="y_sb")
                    for nh in range(2):
                        py = ps_y.tile([P, QW], F32, name="py")
                        for pr in range(2):
                            nc.tensor.matmul(
                                py[:],
                                otn[:, pr, mt * P:(mt + 1) * P],
                                wo_sb[:, pr, nh * QW:(nh + 1) * QW],
                                start=(pr == 0), stop=(pr == 1),
                            )
                        nc.vector.tensor_copy(y_sb[:, nh * QW:(nh + 1) * QW], py[:])
                        yield
                    nc.sync.dma_start(y[mt * P:(mt + 1) * P, :], y_sb[:])

            def attention(pr, qb, fuse_vproj=False, filler=None, filler_start=0):
                qs = slice(qb * QW, (qb + 1) * QW)
                po = [ps_o.tile([HD + 1, QW], F32, tag=f"po{i}", name=f"po{i}")
                      for i in range(2)]
                OT_LAG = 4
                es = {}

                def emit_ot(kt):
                    for hi in range(2):
                        nc.tensor.matmul(
                            po[hi][:],
                            v[:, kt, 2 * pr + hi, :],
                            es[kt][:, hi, :],
                            start=(kt == 0), stop=(kt == NT - 1),
                        )

                for ktile in range(NT):
                    pS = ps_s.tile([P, 2, QW], F32, name="pS")
                    for hi in range(2):
                        lo = hi * HD
                        nc.tensor.matmul(
                            pS[:, hi, :],
                            kts[pr][lo:lo + HD, ktile * P:(ktile + 1) * P],
                            qts[pr][lo:lo + HD, qs],
                            start=True, stop=True,
                        )
                    if fuse_vproj:
                        # v[ktile] is first read by this block's OT matmuls;
                        # emitting its projection here (below the S^T matmuls
                        # in priority) lets the exp stream start ~30us earlier.
                        proj_v(ktile)
                    es[ktile] = epool.tile([P, 2, QW], F16, name="e_sb")
                    nc.scalar.activation(es[ktile][:], pS[:], AF.Exp, scale=SCALE)
                    # filler sits between exp and the (lagged) OT pair: the
                    # in-order PE stream is waiting on exp here, so this is
                    # free PE time. filler_start>0 skips early k-tiles when the
                    # first piece depends on the previous block's normalize.
                    if filler is not None and ktile >= filler_start:
                        next(filler, None)
                    # OTs lag 2 k-tiles: a new block's first STs (which feed the
                    # exp stream) are never queued behind the OT matmuls that
                    # wait on the previous block's accumulator evacuation.
                    if ktile >= OT_LAG:
                        emit_ot(ktile - OT_LAG)
                for kt in range(NT - OT_LAG, NT):
                    emit_ot(kt)
                if filler is not None:
                    for _ in filler:
                        pass
                for hi in range(2):
                    # Evacuate the accumulator to SBUF right away (frees its
                    # PSUM bank for the next block); normalize from the copy.
                    ou = npool.tile([HD + 1, QW], F32, tag="ou", name="ou")
                    nc.vector.tensor_copy(ou[:], po[hi][:])
                    s0 = npool.tile([1, QW], F32, tag="s0", name="s0")
                    nc.sync.dma_start(s0[:], ou[HD:HD + 1, :])
                    r0 = npool.tile([1, QW], F32, tag="r0", name="r0")
                    nc.vector.reciprocal_approx_fast(r0[:], s0[:])
                    rb = npool.tile([HD, QW], F32, tag="rb", name="rb")
                    nc.gpsimd.partition_broadcast(rb[:], r0[:])
                    if hi == 0:
                        nc.vector.tensor_mul(
                            out=otn[0:HD, pr, qs], in0=ou[0:HD, :], in1=rb[:])
                    else:
                        tmp = npool.tile([HD, QW], F16, tag="tmp", name="tmp")
                        nc.vector.tensor_mul(out=tmp[:], in0=ou[0:HD, :], in1=rb[:])
                        nc.sync.dma_start(otn[HD:P, pr, qs], tmp[:])

            def outproj(mt, ps_y):
                y_sb = ypool.tile([P, D], F32, name="y_sb")
                for nh in range(2):
                    py = ps_y.tile([P, QW], F32, name="py")
                    for pr in range(2):
                        nc.tensor.matmul(
                            py[:],
                            otn[:, pr, mt * P:(mt + 1) * P],
                            wo_sb[:, pr, nh * QW:(nh + 1) * QW],
                            start=(pr == 0), stop=(pr == 1),
                        )
                    nc.vector.tensor_copy(y_sb[:, nh * QW:(nh + 1) * QW], py[:])
                nc.sync.dma_start(y[mt * P:(mt + 1) * P, :], y_sb[:])

            # ---- schedule: pair-0 K/Q first so the exp stream starts early
            # (S^T needs only K/Q; the deep E pool lets exp run ahead of the
            # V-dependent OT matmuls while the PE catches up on V). Pair-1
            # projections are emitted between pair-0 attention blocks to fill
            # the exp stream's PE idle. NOTE: Tile dependencies follow
            # emission order, so every tensor must be emitted-written before
            # an emitted-read (no read-before-write reordering).
            for tcb in range(NQB):
                proj_qk(wk_sb, bk_sb, kts, 0, tcb)
                proj_qk(wq_sb, bq_sb, qts, 0, tcb)
            attention(0, 0, fuse_vproj=True)
            attention(0, 1, filler=proj_qk_gen(wk_sb, bk_sb, kts, 1))
            attention(0, 2, filler=proj_qk_gen(wq_sb, bq_sb, qts, 1))
            attention(0, 3)
            _psP.close()
            with tc.tile_pool(name="ps_y", bufs=2, space="PSUM") as ps_y:
                # out-projection for q-block qb fills the next attention
                # block's exp-paced PE idle.
                attention(1, 0)
                attention(1, 1, filler=outproj_gen(range(0, 4), ps_y), filler_start=4)
                attention(1, 2, filler=outproj_gen(range(4, 8), ps_y), filler_start=4)
                attention(1, 3, filler=outproj_gen(range(8, 12), ps_y), filler_start=4)
                for _ in outproj_gen(range(12, NT), ps_y):
                    pass

    nc.compile()
    return nc


def _get_nc():
    if "nc" not in _CACHE:
        _CACHE["nc"] = _build()
    return _CACHE["nc"]


def kernel(x, W_qkv, b_qkv, W_out, b_out):
    x = np.asarray(x, dtype=np.float32)
    W_qkv16 = np.asarray(W_qkv, dtype=np.float32).astype(np.float16)
    b_qkv = np.asarray(b_qkv, dtype=np.float32)
    W_out16 = np.asarray(W_out, dtype=np.float32).astype(np.float16)
    b_out = np.asarray(b_out, dtype=np.float32)

    nc = _get_nc()

    def pack_w(w2d):  # [K, C] -> [P, K//P, C]
        return np.ascontiguousarray(
            w2d.reshape(w2d.shape[0] // P, P, -1).transpose(1, 0, 2))

    xt16s = [np.ascontiguousarray(
        x[b].astype(np.float16).reshape(NQB, QW, DO, P).transpose(0, 3, 2, 1))
        for b in range(B)]
    in_maps = []
    for core in range(NCORES):
        b = core // 4
        c0 = (core % 4) * GH
        in_maps.append({
            "xt16": xt16s[b],
            "wq": pack_w(W_qkv16[:, c0:c0 + GH]),
            "wk": pack_w(W_qkv16[:, D + c0:D + c0 + GH]),
            "wv": pack_w(W_qkv16[:, 2 * D + c0:2 * D + c0 + GH]),
            "wo": pack_w(W_out16[c0:c0 + GH, :]),
            "bq": np.ascontiguousarray(b_qkv[c0:c0 + GH].reshape(2, P).T),
            "bk": np.ascontiguousarray(b_qkv[D + c0:D + c0 + GH].reshape(2, P).T),
            "bv": np.ascontiguousarray(b_qkv[2 * D + c0:2 * D + c0 + GH]),
        })

    import os
    trace = bool(int(os.environ.get("BASS_KERNEL_TRACE", "0")))
    res = run_bass_kernel_spmd(nc, in_maps, list(range(NCORES)), trace=trace)
    _CACHE["last_result"] = res

    parts = [res.results[i]["y"] for i in range(NCORES)]
    out = np.empty((B, L, D), dtype=np.float32)
    out[0] = parts[0] + parts[1] + parts[2] + parts[3]
    out[1] = parts[4] + parts[5] + parts[6] + parts[7]
    out += b_out
    return out


# revision 50
# speedup vs baseline: 1.0928x; 1.0164x over previous
"""Multi-head self-attention (B=2, L=2048, D=1024, H=16) on 8 Trainium2 cores.

Sharding: batch x head-group. Core c handles batch c//4 and heads
(c%4)*4 .. (c%4)*4+4. Each core computes Q/K/V projections for its 4 heads,
attention, and a partial out-projection (row-slice of W_out); the host sums
the 4 partials per batch and adds b_out.

The device pipeline runs in fp16 (fp32 PSUM accumulation everywhere): fp16
matmuls stream at 1 cycle/row on the PE vs 2 for fp32r, and 2-byte dtypes get
the XBAR DMA-transpose path so x is transposed during the load.

Device layouts (per core):
  XT  [128d, 8do, 2048t]  x[b]^T via DMA transpose
  QT/KT [128f, 2pair, 2048t] head-pair features on partitions
  V   [128t, 16mt, 4h, 65]  per-token values; col 64 is ones so the attention
                            matmul also emits softmax denominators
  OTn [128hd, 2pair, 2048t] normalized attention output, pair-stacked (odd
                            head shifted to partitions 64:128 via SBUF DMA)

Attention per (pair, q-block of 512), streaming 16 k-tiles:
  S^T(h0), S^T(h1) -> one 2-bank PSUM tile; one exp ACTIVATE over both banks
  (scale=1/8 folded in; logits are O(1) so no max-subtraction is needed)
  -> E [128, 2, 512] fp16; OT_h += [V_h|1].T @ E_h (PSUM accumulation).
The OT accumulators are copied to SBUF immediately (releasing their PSUM
banks) and normalized there: r = 1/OT[64] via reciprocal_approx_fast at
partition 0, gpsimd partition-broadcast, OTn = OT[:64] * r.

The attention stream is ScalarE(exp)-bound, so everything else (V projection,
pair-1 Q/K projections, out-projection) is emitted *between* attention blocks:
Tile's priority scheduler slots that PE work into the exp stream's PE idle.
PSUM: 4 banks S^T (double-buffered 2-bank tiles) + 2 banks OT accumulators +
2 banks for projection/out-projection accumulators.
"""

import sys

for _p in ("/opt/trn_rl_repo", "/opt/pypackages"):
    if _p not in sys.path:
        sys.path.insert(0, _p)

import numpy as np

import concourse.bacc as bacc
import concourse.mybir as mybir
import concourse.tile as tile
from concourse.bass_utils import run_bass_kernel_spmd

F32 = mybir.dt.float32
F16 = mybir.dt.float16
AF = mybir.ActivationFunctionType

B, L, D = 2, 2048, 1024
NH, HD = 16, 64
P = 128
NCORES = 8
NHL = 4                 # heads per core
GH = NHL * HD           # 256: per-core feature-group width
DO = D // P             # 8 contraction subtiles
NT = L // P             # 16 token tiles
NQB = 4                 # q blocks
QW = L // NQB           # 512
SCALE = float(HD) ** -0.5

_CACHE = {}


def _build():
    nc = bacc.Bacc("TRN2", target_bir_lowering=False, debug=False)

    # all inputs arrive pre-packed in device layout (partition-major with
    # multi-KB contiguous per-partition lines -> full DMA bandwidth)
    xt16 = nc.dram_tensor("xt16", [NQB, P, DO, QW], F16, kind="ExternalInput")
    wq = nc.dram_tensor("wq", [P, DO, GH], F16, kind="ExternalInput")
    wk = nc.dram_tensor("wk", [P, DO, GH], F16, kind="ExternalInput")
    wv = nc.dram_tensor("wv", [P, DO, GH], F16, kind="ExternalInput")
    wo = nc.dram_tensor("wo", [P, 2, D], F16, kind="ExternalInput")
    bq = nc.dram_tensor("bq", [P, 2], F32, kind="ExternalInput")
    bk = nc.dram_tensor("bk", [P, 2], F32, kind="ExternalInput")
    bv = nc.dram_tensor("bv", [GH], F32, kind="ExternalInput")
    y = nc.dram_tensor("y", [L, D], F32, kind="ExternalOutput")

    with tile.TileContext(nc) as tc:
        with tc.tile_pool(name="persist", bufs=1) as persist, \
             tc.tile_pool(name="wpool", bufs=1) as wpool, \
             tc.tile_pool(name="xt_pool", bufs=1) as xt_pool, \
             tc.tile_pool(name="epool", bufs=18) as epool, \
             tc.tile_pool(name="npool", bufs=4) as npool, \
             tc.tile_pool(name="ypool", bufs=3) as ypool, \
             tc.tile_pool(name="ps_s", bufs=2, space="PSUM") as ps_s, \
             tc.tile_pool(name="ps_o", bufs=1, space="PSUM") as ps_o:
            from contextlib import ExitStack
            _psP = ExitStack()
            ps_p = _psP.enter_context(tc.tile_pool(name="ps_p", bufs=2, space="PSUM"))

            # ---- PE warmup: ~10us of zero matmuls so HAM un-throttles while
            # the input DMAs stream in.
            wz = persist.tile([P, QW], F16)
            nc.vector.memset(wz[:], 0.0)
            pwm = ps_p.tile([P, QW], F32, tag="pj", name="pwm")
            for _ in range(14):
                nc.tensor.matmul(pwm[:], wz[:, 0:P], wz[:], start=True, stop=True)

            qt = persist.tile([P, 2, L], F16)
            kt = persist.tile([P, 2, L], F16)
            qts = [qt[:, i, :] for i in range(2)]
            kts = [kt[:, i, :] for i in range(2)]
            v = persist.tile([P, NT, NHL, HD + 1], F16)
            otn = persist.tile([P, 2, L], F16)

            # x arrives pre-transposed from the host; stream it in chunks on
            # the sync HWDGE queue (first token block in per-do pieces so the
            # first K-projection group can start ASAP) while the scalar HWDGE
            # queue loads the weights in parallel.
            xts = [xt_pool.tile([P, DO, QW], F16, name=f"xt{i}", tag=f"xt{i}")
                   for i in range(NQB)]
            def xt_chunk(q, tcb):
                q.dma_start(xts[tcb][:], xt16[tcb])

            wk_sb = wpool.tile([P, DO, GH], F16)
            wq_sb = wpool.tile([P, DO, GH], F16)
            wv_sb = wpool.tile([P, DO, GH], F16)
            wo_sb = wpool.tile([P, 2, D], F16)
            bq_sb = persist.tile([P, 2], F32)
            bk_sb = persist.tile([P, 2], F32)
            bv_row = persist.tile([1, GH], F32)
            # NOTE: never issue DMAs on the scalar queue — the exp ACTIVATEs
            # would queue behind them (strict FIFO) and the whole attention
            # stream is exp-paced. Bulk loads go on sync; gpsimd (SWDGE)
            # carries the later chunks in parallel.
            # single queue => in-order service at full HBM bandwidth, in
            # critical-path order (the load is HBM-bound at ~350GB/s).
            nc.sync.dma_start(wk_sb[:], wk[:])
            nc.sync.dma_start(wq_sb[:], wq[:])
            xt_chunk(nc.sync, 0)
            xt_chunk(nc.sync, 1)
            xt_chunk(nc.sync, 2)
            xt_chunk(nc.sync, 3)
            nc.sync.dma_start(wv_sb[:], wv[:])
            nc.sync.dma_start(wo_sb[:], wo[:])
            nc.gpsimd.dma_start(bq_sb[:], bq[:])
            nc.gpsimd.dma_start(bk_sb[:], bk[:])
            nc.gpsimd.dma_start(bv_row[:], bv[None, :])
            bv_sb = persist.tile([P, GH], F32)
            nc.gpsimd.partition_broadcast(bv_sb[:], bv_row[:])
            # ones column for the denominator rows
            ones_sb = persist.tile([P, 1], F32)
            nc.vector.memset(ones_sb[:], 1.0)
            nc.vector.tensor_copy(
                v[:, :, :, HD], ones_sb[:, 0:1, None].to_broadcast((P, NT, NHL)))

            def proj_v(mt):
                pv = ps_p.tile([P, QW], F32, tag="pj", name="pv")
                for do in range(DO):
                    nc.tensor.matmul(
                        pv[:, 0:GH],
                        xts[mt // 4][:, do, (mt % 4) * P:(mt % 4 + 1) * P],
                        wv_sb[:, do, :],
                        start=(do == 0), stop=(do == DO - 1),
                    )
                nc.vector.tensor_add(
                    out=v[:, mt, :, 0:HD],
                    in0=pv[:, 0:GH].rearrange("p (h c) -> p h c", h=NHL),
                    in1=bv_sb[:].rearrange("p (h c) -> p h c", h=NHL),
                )

            def proj_qk(w_sb, b_sb, dst, pr, tcb):
                pj = ps_p.tile([P, QW], F32, tag="pj", name="pj")
                for do in range(DO):
                    nc.tensor.matmul(
                        pj[:],
                        w_sb[:, do, pr * P:(pr + 1) * P],
                        xts[tcb][:, do, :],
                        start=(do == 0), stop=(do == DO - 1),
                    )
                nc.vector.tensor_scalar_add(
                    dst[pr][:, tcb * QW:(tcb + 1) * QW], pj[:], b_sb[:, pr:pr + 1])

            def proj_qk_gen(w_sb, b_sb, dst, pr):
                # generator form: 2 matmuls per step, one step per attention
                # k-tile, so the projection never delays the next S^T pair.
                for tcb in range(NQB):
                    pj = ps_p.tile([P, QW], F32, tag="pj", name="pj")
                    for do in range(DO):
                        nc.tensor.matmul(
                            pj[:],
                            w_sb[:, do, pr * P:(pr + 1) * P],
                            xts[tcb][:, do, :],
                            start=(do == 0), stop=(do == DO - 1),
                        )
                        if do % 2 == 1:
                            yield
                    nc.vector.tensor_scalar_add(
                        dst[pr][:, tcb * QW:(tcb + 1) * QW], pj[:], b_sb[:, pr:pr + 1])

            def outproj_gen(mts, ps_y):
                for mt in mts:
                    y_sb = ypool.tile([P, D], F32, name="y_sb")
                    for nh in range(2):
                        py = ps_y.tile([P, QW], F32, name="py")
                        for pr in range(2):
                            nc.tensor.matmul(
                                py[:],
                                otn[:, pr, mt * P:(mt + 1) * P],
                                wo_sb[:, pr, nh * QW:(nh + 1) * QW],
                                start=(pr == 0), stop=(pr == 1),
                            )
                        nc.vector.tensor_copy(y_sb[:, nh * QW:(nh + 1) * QW], py[:])
                        yield
                    nc.sync.dma_start(y[mt * P:(mt + 1) * P, :], y_sb[:])

            def attention(pr, qb, fuse_vproj=False, filler=None, filler_start=0):
                qs = slice(qb * QW, (qb + 1) * QW)
                po = [ps_o.tile([HD + 1, QW], F32, tag=f"po{i}", name=f"po{i}")
                      for i in range(2)]
                OT_LAG = 4
                es = {}

                def emit_ot(kt):
                    for hi in range(2):
                        nc.tensor.matmul(
                            po[hi][:],
                            v[:, kt, 2 * pr + hi, :],
                            es[kt][:, hi, :],
                            start=(kt == 0), stop=(kt == NT - 1),
                        )

                for ktile in range(NT):
                    pS = ps_s.tile([P, 2, QW], F32, name="pS")
                    for hi in range(2):
                        lo = hi * HD
                        nc.tensor.matmul(
                            pS[:, hi, :],
                            kts[pr][lo:lo + HD, ktile * P:(ktile + 1) * P],
                            qts[pr][lo:lo + HD, qs],
                            start=True, stop=True,
                        )
                    if fuse_vproj:
                        # v[ktile] is first read by this block's OT matmuls;
                        # emitting its projection here (below the S^T matmuls
                        # in priority) lets the exp stream start ~30us earlier.
                        proj_v(ktile)
                    es[ktile] = epool.tile([P, 2, QW], F16, name="e_sb")
                    nc.scalar.activation(es[ktile][:], pS[:], AF.Exp, scale=SCALE)
                    # filler sits between exp and the (lagged) OT pair: the
                    # in-order PE stream is waiting on exp here, so this is
                    # free PE time. filler_start>0 skips early k-tiles when the
                    # first piece depends on the previous block's normalize.
                    if filler is not None and ktile >= filler_start:
                        next(filler, None)
                    # OTs lag 2 k-tiles: a new block's first STs (which feed the
                    # exp stream) are never queued behind the OT matmuls that
                    # wait on the previous block's accumulator evacuation.
                    if ktile >= OT_LAG:
                        emit_ot(ktile - OT_LAG)
                for kt in range(NT - OT_LAG, NT):
                    emit_ot(kt)
                if filler is not None:
                    for _ in filler:
                        pass
                for hi in range(2):
                    # Evacuate the accumulator to SBUF right away (frees its
                    # PSUM bank for the next block); normalize from the copy.
                    ou = npool.tile([HD + 1, QW], F32, tag="ou", name="ou")
                    nc.vector.tensor_copy(ou[:], po[hi][:])
                    s0 = npool.tile([1, QW], F32, tag="s0", name="s0")
                    nc.sync.dma_start(s0[:], ou[HD:HD + 1, :])
                    r0 = npool.tile([1, QW], F32, tag="r0", name="r0")
                    nc.vector.reciprocal_approx_fast(r0[:], s0[:])
                    rb = npool.tile([HD, QW], F32, tag="rb", name="rb")
                    nc.gpsimd.partition_broadcast(rb[:], r0[:])
                    if hi == 0:
                        nc.vector.tensor_mul(
                            out=otn[0:HD, pr, qs], in0=ou[0:HD, :], in1=rb[:])
                    else:
                        tmp = npool.tile([HD, QW], F16, tag="tmp", name="tmp")
                        nc.vector.tensor_mul(out=tmp[:], in0=ou[0:HD, :], in1=rb[:])
                        nc.sync.dma_start(otn[HD:P, pr, qs], tmp[:])

            def outproj(mt, ps_y):
                y_sb = ypool.tile([P, D], F32, name="y_sb")
                for nh in range(2):
                    py = ps_y.tile([P, QW], F32, name="py")
                    for pr in range(2):
                        nc.tensor.matmul(
                            py[:],
                            otn[:, pr, mt * P:(mt + 1) * P],
                            wo_sb[:, pr, nh * QW:(nh + 1) * QW],
                            start=(pr == 0), stop=(pr == 1),
                        )
                    nc.vector.tensor_copy(y_sb[:, nh * QW:(nh + 1) * QW], py[:])
                nc.sync.dma_start(y[mt * P:(mt + 1) * P, :], y_sb[:])

            # ---- schedule: pair-0 K/Q first so the exp stream starts early
            # (S^T needs only K/Q; the deep E pool lets exp run ahead of the
            # V-dependent OT matmuls while the PE catches up on V). Pair-1
            # projections are emitted between pair-0 attention blocks to fill
            # the exp stream's PE idle. NOTE: Tile dependencies follow
            # emission order, so every tensor must be emitted-written before
            # an emitted-read (no read-before-write reordering).
            for tcb in range(NQB):
                proj_qk(wk_sb, bk_sb, kts, 0, tcb)
                proj_qk(wq_sb, bq_sb, qts, 0, tcb)
            attention(0, 0, fuse_vproj=True)
            attention(0, 1, filler=proj_qk_gen(wk_sb, bk_sb, kts, 1))
            attention(0, 2, filler=proj_qk_gen(wq_sb, bq_sb, qts, 1))
            attention(0, 3)
            _psP.close()
            with tc.tile_pool(name="ps_y", bufs=2, space="PSUM") as ps_y:
                # out-projection for q-block qb fills the next attention
                # block's exp-paced PE idle.
                attention(1, 0)
                attention(1, 1, filler=outproj_gen(range(0, 4), ps_y), filler_start=6)
                attention(1, 2, filler=outproj_gen(range(4, 8), ps_y), filler_start=6)
                attention(1, 3, filler=outproj_gen(range(8, 12), ps_y), filler_start=6)
                for _ in outproj_gen(range(12, NT), ps_y):
                    pass

    nc.compile()
    return nc


def _get_nc():
    if "nc" not in _CACHE:
        _CACHE["nc"] = _build()
    return _CACHE["nc"]


def kernel(x, W_qkv, b_qkv, W_out, b_out):
    x = np.asarray(x, dtype=np.float32)
    W_qkv16 = np.asarray(W_qkv, dtype=np.float32).astype(np.float16)
    b_qkv = np.asarray(b_qkv, dtype=np.float32)
    W_out16 = np.asarray(W_out, dtype=np.float32).astype(np.float16)
    b_out = np.asarray(b_out, dtype=np.float32)

    nc = _get_nc()

    def pack_w(w2d):  # [K, C] -> [P, K//P, C]
        return np.ascontiguousarray(
            w2d.reshape(w2d.shape[0] // P, P, -1).transpose(1, 0, 2))

    xt16s = [np.ascontiguousarray(
        x[b].astype(np.float16).reshape(NQB, QW, DO, P).transpose(0, 3, 2, 1))
        for b in range(B)]
    in_maps = []
    for core in range(NCORES):
        b = core // 4
        c0 = (core % 4) * GH
        in_maps.append({
            "xt16": xt16s[b],
            "wq": pack_w(W_qkv16[:, c0:c0 + GH]),
            "wk": pack_w(W_qkv16[:, D + c0:D + c0 + GH]),
            "wv": pack_w(W_qkv16[:, 2 * D + c0:2 * D + c0 + GH]),
            "wo": pack_w(W_out16[c0:c0 + GH, :]),
            "bq": np.ascontiguousarray(b_qkv[c0:c0 + GH].reshape(2, P).T),
            "bk": np.ascontiguousarray(b_qkv[D + c0:D + c0 + GH].reshape(2, P).T),
            "bv": np.ascontiguousarray(b_qkv[2 * D + c0:2 * D + c0 + GH]),
        })

    import os
    trace = bool(int(os.environ.get("BASS_KERNEL_TRACE", "0")))
    res = run_bass_kernel_spmd(nc, in_maps, list(range(NCORES)), trace=trace)
    _CACHE["last_result"] = res

    parts = [res.results[i]["y"] for i in range(NCORES)]
    out = np.empty((B, L, D), dtype=np.float32)
    out[0] = parts[0] + parts[1] + parts[2] + parts[3]
    out[1] = parts[4] + parts[5] + parts[6] + parts[7]
    out += b_out
    return out


# revision 51
# speedup vs baseline: 1.0949x; 1.0019x over previous
"""Multi-head self-attention (B=2, L=2048, D=1024, H=16) on 8 Trainium2 cores.

Sharding: batch x head-group. Core c handles batch c//4 and heads
(c%4)*4 .. (c%4)*4+4. Each core computes Q/K/V projections for its 4 heads,
attention, and a partial out-projection (row-slice of W_out); the host sums
the 4 partials per batch and adds b_out.

The device pipeline runs in fp16 (fp32 PSUM accumulation everywhere): fp16
matmuls stream at 1 cycle/row on the PE vs 2 for fp32r. All inputs are
pre-packed on the host into the exact device layouts (x pre-transposed,
partition-major, multi-KB contiguous per-partition lines so the input load
runs at full HBM bandwidth) and streamed on one HWDGE queue in critical-path
order. The scalar queue carries no DMAs - exp ACTIVATEs would queue behind
them.

Device layouts (per core):
  XT  4x [128d, 8do, 512t]  x[b]^T token-block chunks
  QT/KT [128f, 2048t] per head-pair (2 heads x 64 features on partitions)
  V   [128t, 16mt, 4h, 65]  per-token values; col 64 is ones so the attention
                            matmul also emits softmax denominators
  OTn [128hd, 2pair, 2048t] normalized attention output, pair-stacked (odd
                            head shifted to partitions 64:128 via SBUF DMA)

Attention per (pair, q-block of 512), streaming 16 k-tiles:
  S^T(h0), S^T(h1) -> one 2-bank PSUM tile; one exp ACTIVATE over both banks
  (scale=1/8 folded in; logits are O(1) so no max-subtraction is needed)
  -> E [128, 2, 512] fp16; OT_h += [V_h|1].T @ E_h (PSUM accumulation).
The OT accumulators are copied to SBUF immediately (releasing their PSUM
banks) and normalized there: r = 1/OT[64] via reciprocal_approx_fast at
partition 0, gpsimd partition-broadcast, OTn = OT[:64] * r.

The attention stream is ScalarE(exp)-paced, and each engine's instruction
stream executes in order, so scheduling is about keeping the exp stream fed:
S^T matmuls lead, OT matmuls lag 4 k-tiles (they may wait on accumulator
evacuation), and all other PE work (V projection, pair-1 Q/K projections,
out-projection) is spliced between exp and the lagged OTs as fine-grained
generator "fillers" that run in the PE's exp-wait windows. PE warmup matmuls
at the start un-throttle the HAM clock gate during the input load.
PSUM: 4 banks S^T (double-buffered 2-bank tiles) + 2 banks OT accumulators +
2 banks for projection/out-projection accumulators.
"""

import sys

for _p in ("/opt/trn_rl_repo", "/opt/pypackages"):
    if _p not in sys.path:
        sys.path.insert(0, _p)

import numpy as np

import concourse.bacc as bacc
import concourse.mybir as mybir
import concourse.tile as tile
from concourse.bass_utils import run_bass_kernel_spmd

F32 = mybir.dt.float32
F16 = mybir.dt.float16
AF = mybir.ActivationFunctionType

B, L, D = 2, 2048, 1024
NH, HD = 16, 64
P = 128
NCORES = 8
NHL = 4                 # heads per core
GH = NHL * HD           # 256: per-core feature-group width
DO = D // P             # 8 contraction subtiles
NT = L // P             # 16 token tiles
NQB = 4                 # q blocks
QW = L // NQB           # 512
SCALE = float(HD) ** -0.5

_CACHE = {}


def _build():
    nc = bacc.Bacc("TRN2", target_bir_lowering=False, debug=False)

    # all inputs arrive pre-packed in device layout (partition-major with
    # multi-KB contiguous per-partition lines -> full DMA bandwidth)
    xt16 = nc.dram_tensor("xt16", [NQB, P, DO, QW], F16, kind="ExternalInput")
    wq = nc.dram_tensor("wq", [P, DO, GH], F16, kind="ExternalInput")
    wk = nc.dram_tensor("wk", [P, DO, GH], F16, kind="ExternalInput")
    wv = nc.dram_tensor("wv", [P, DO, GH], F16, kind="ExternalInput")
    wo = nc.dram_tensor("wo", [P, 2, D], F16, kind="ExternalInput")
    bq = nc.dram_tensor("bq", [P, 2], F32, kind="ExternalInput")
    bk = nc.dram_tensor("bk", [P, 2], F32, kind="ExternalInput")
    bv = nc.dram_tensor("bv", [GH], F32, kind="ExternalInput")
    y = nc.dram_tensor("y", [L, D], F32, kind="ExternalOutput")

    with tile.TileContext(nc) as tc:
        with tc.tile_pool(name="persist", bufs=1) as persist, \
             tc.tile_pool(name="wpool", bufs=1) as wpool, \
             tc.tile_pool(name="xt_pool", bufs=1) as xt_pool, \
             tc.tile_pool(name="epool", bufs=18) as epool, \
             tc.tile_pool(name="npool", bufs=4) as npool, \
             tc.tile_pool(name="ypool", bufs=3) as ypool, \
             tc.tile_pool(name="ps_s", bufs=2, space="PSUM") as ps_s, \
             tc.tile_pool(name="ps_o", bufs=1, space="PSUM") as ps_o:
            from contextlib import ExitStack
            _psP = ExitStack()
            ps_p = _psP.enter_context(tc.tile_pool(name="ps_p", bufs=2, space="PSUM"))

            # ---- PE warmup: ~10us of zero matmuls so HAM un-throttles while
            # the input DMAs stream in.
            wz = persist.tile([P, QW], F16)
            nc.vector.memset(wz[:], 0.0)
            pwm = ps_p.tile([P, QW], F32, tag="pj", name="pwm")
            for _ in range(14):
                nc.tensor.matmul(pwm[:], wz[:, 0:P], wz[:], start=True, stop=True)

            qt = persist.tile([P, 2, L], F16)
            kt = persist.tile([P, 2, L], F16)
            qts = [qt[:, i, :] for i in range(2)]
            kts = [kt[:, i, :] for i in range(2)]
            v = persist.tile([P, NT, NHL, HD + 1], F16)
            otn = persist.tile([P, 2, L], F16)

            # x arrives pre-transposed and pre-packed from the host.
            xts = [xt_pool.tile([P, DO, QW], F16, name=f"xt{i}", tag=f"xt{i}")
                   for i in range(NQB)]
            def xt_chunk(q, tcb):
                q.dma_start(xts[tcb][:], xt16[tcb])

            wk_sb = wpool.tile([P, DO, GH], F16)
            wq_sb = wpool.tile([P, DO, GH], F16)
            wv_sb = wpool.tile([P, DO, GH], F16)
            wo_sb = wpool.tile([P, 2, D], F16)
            bq_sb = persist.tile([P, 2], F32)
            bk_sb = persist.tile([P, 2], F32)
            bv_row = persist.tile([1, GH], F32)
            # NOTE: never issue DMAs on the scalar queue — the exp ACTIVATEs
            # would queue behind them (strict FIFO) and the whole attention
            # stream is exp-paced. Bulk loads go on sync; gpsimd (SWDGE)
            # carries the later chunks in parallel.
            # single queue => in-order service at full HBM bandwidth, in
            # critical-path order (the load is HBM-bound at ~350GB/s).
            nc.sync.dma_start(wk_sb[:], wk[:])
            nc.sync.dma_start(wq_sb[:], wq[:])
            xt_chunk(nc.sync, 0)
            xt_chunk(nc.sync, 1)
            xt_chunk(nc.sync, 2)
            xt_chunk(nc.sync, 3)
            nc.sync.dma_start(wv_sb[:], wv[:])
            nc.sync.dma_start(wo_sb[:], wo[:])
            nc.gpsimd.dma_start(bq_sb[:], bq[:])
            nc.gpsimd.dma_start(bk_sb[:], bk[:])
            nc.gpsimd.dma_start(bv_row[:], bv[None, :])
            bv_sb = persist.tile([P, GH], F32)
            nc.gpsimd.partition_broadcast(bv_sb[:], bv_row[:])
            # ones column for the denominator rows
            ones_sb = persist.tile([P, 1], F32)
            nc.vector.memset(ones_sb[:], 1.0)
            nc.vector.tensor_copy(
                v[:, :, :, HD], ones_sb[:, 0:1, None].to_broadcast((P, NT, NHL)))

            def proj_v(mt):
                pv = ps_p.tile([P, QW], F32, tag="pj", name="pv")
                for do in range(DO):
                    nc.tensor.matmul(
                        pv[:, 0:GH],
                        xts[mt // 4][:, do, (mt % 4) * P:(mt % 4 + 1) * P],
                        wv_sb[:, do, :],
                        start=(do == 0), stop=(do == DO - 1),
                    )
                nc.vector.tensor_add(
                    out=v[:, mt, :, 0:HD],
                    in0=pv[:, 0:GH].rearrange("p (h c) -> p h c", h=NHL),
                    in1=bv_sb[:].rearrange("p (h c) -> p h c", h=NHL),
                )

            def proj_qk(w_sb, b_sb, dst, pr, tcb):
                pj = ps_p.tile([P, QW], F32, tag="pj", name="pj")
                for do in range(DO):
                    nc.tensor.matmul(
                        pj[:],
                        w_sb[:, do, pr * P:(pr + 1) * P],
                        xts[tcb][:, do, :],
                        start=(do == 0), stop=(do == DO - 1),
                    )
                nc.vector.tensor_scalar_add(
                    dst[pr][:, tcb * QW:(tcb + 1) * QW], pj[:], b_sb[:, pr:pr + 1])

            def proj_qk_gen(w_sb, b_sb, dst, pr):
                # generator form: 2 matmuls per step, one step per attention
                # k-tile, so the projection never delays the next S^T pair.
                for tcb in range(NQB):
                    pj = ps_p.tile([P, QW], F32, tag="pj", name="pj")
                    for do in range(DO):
                        nc.tensor.matmul(
                            pj[:],
                            w_sb[:, do, pr * P:(pr + 1) * P],
                            xts[tcb][:, do, :],
                            start=(do == 0), stop=(do == DO - 1),
                        )
                        if do % 2 == 1:
                            yield
                    nc.vector.tensor_scalar_add(
                        dst[pr][:, tcb * QW:(tcb + 1) * QW], pj[:], b_sb[:, pr:pr + 1])

            def outproj_gen(mts, ps_y):
                for mt in mts:
                    y_sb = ypool.tile([P, D], F32, name="y_sb")
                    for nh in range(2):
                        py = ps_y.tile([P, QW], F32, name="py")
                        for pr in range(2):
                            nc.tensor.matmul(
                                py[:],
                                otn[:, pr, mt * P:(mt + 1) * P],
                                wo_sb[:, pr, nh * QW:(nh + 1) * QW],
                                start=(pr == 0), stop=(pr == 1),
                            )
                        nc.vector.tensor_copy(y_sb[:, nh * QW:(nh + 1) * QW], py[:])
                        yield
                    nc.sync.dma_start(y[mt * P:(mt + 1) * P, :], y_sb[:])

            def attention(pr, qb, fuse_vproj=False, filler=None, filler_start=0):
                qs = slice(qb * QW, (qb + 1) * QW)
                po = [ps_o.tile([HD + 1, QW], F32, tag=f"po{i}", name=f"po{i}")
                      for i in range(2)]
                OT_LAG = 4
                es = {}

                def emit_ot(kt):
                    for hi in range(2):
                        nc.tensor.matmul(
                            po[hi][:],
                            v[:, kt, 2 * pr + hi, :],
                            es[kt][:, hi, :],
                            start=(kt == 0), stop=(kt == NT - 1),
                        )

                for ktile in range(NT):
                    pS = ps_s.tile([P, 2, QW], F32, name="pS")
                    for hi in range(2):
                        lo = hi * HD
                        nc.tensor.matmul(
                            pS[:, hi, :],
                            kts[pr][lo:lo + HD, ktile * P:(ktile + 1) * P],
                            qts[pr][lo:lo + HD, qs],
                            start=True, stop=True,
                        )
                    if fuse_vproj:
                        # v[ktile] is first read by this block's OT matmuls;
                        # emitting its projection here (below the S^T matmuls
                        # in priority) lets the exp stream start ~30us earlier.
                        proj_v(ktile)
                    es[ktile] = epool.tile([P, 2, QW], F16, name="e_sb")
                    nc.scalar.activation(es[ktile][:], pS[:], AF.Exp, scale=SCALE)
                    # filler sits between exp and the (lagged) OT pair: the
                    # in-order PE stream is waiting on exp here, so this is
                    # free PE time. filler_start>0 skips early k-tiles when the
                    # first piece depends on the previous block's normalize.
                    if filler is not None and ktile >= filler_start:
                        next(filler, None)
                    # OTs lag 2 k-tiles: a new block's first STs (which feed the
                    # exp stream) are never queued behind the OT matmuls that
                    # wait on the previous block's accumulator evacuation.
                    if ktile >= OT_LAG:
                        emit_ot(ktile - OT_LAG)
                for kt in range(NT - OT_LAG, NT):
                    emit_ot(kt)
                if filler is not None:
                    for _ in filler:
                        pass
                for hi in range(2):
                    # Evacuate the accumulator to SBUF right away (frees its
                    # PSUM bank for the next block); normalize from the copy.
                    ou = npool.tile([HD + 1, QW], F32, tag="ou", name="ou")
                    nc.vector.tensor_copy(ou[:], po[hi][:])
                    s0 = npool.tile([1, QW], F32, tag="s0", name="s0")
                    nc.sync.dma_start(s0[:], ou[HD:HD + 1, :])
                    r0 = npool.tile([1, QW], F32, tag="r0", name="r0")
                    nc.vector.reciprocal_approx_fast(r0[:], s0[:])
                    rb = npool.tile([HD, QW], F32, tag="rb", name="rb")
                    nc.gpsimd.partition_broadcast(rb[:], r0[:])
                    if hi == 0:
                        nc.vector.tensor_mul(
                            out=otn[0:HD, pr, qs], in0=ou[0:HD, :], in1=rb[:])
                    else:
                        tmp = npool.tile([HD, QW], F16, tag="tmp", name="tmp")
                        nc.vector.tensor_mul(out=tmp[:], in0=ou[0:HD, :], in1=rb[:])
                        nc.sync.dma_start(otn[HD:P, pr, qs], tmp[:])

            def outproj(mt, ps_y):
                y_sb = ypool.tile([P, D], F32, name="y_sb")
                for nh in range(2):
                    py = ps_y.tile([P, QW], F32, name="py")
                    for pr in range(2):
                        nc.tensor.matmul(
                            py[:],
                            otn[:, pr, mt * P:(mt + 1) * P],
                            wo_sb[:, pr, nh * QW:(nh + 1) * QW],
                            start=(pr == 0), stop=(pr == 1),
                        )
                    nc.vector.tensor_copy(y_sb[:, nh * QW:(nh + 1) * QW], py[:])
                nc.sync.dma_start(y[mt * P:(mt + 1) * P, :], y_sb[:])

            # ---- schedule: pair-0 K/Q first so the exp stream starts early
            # (S^T needs only K/Q; the deep E pool lets exp run ahead of the
            # V-dependent OT matmuls while the PE catches up on V). Pair-1
            # projections are emitted between pair-0 attention blocks to fill
            # the exp stream's PE idle. NOTE: Tile dependencies follow
            # emission order, so every tensor must be emitted-written before
            # an emitted-read (no read-before-write reordering).
            for tcb in range(NQB):
                proj_qk(wk_sb, bk_sb, kts, 0, tcb)
                proj_qk(wq_sb, bq_sb, qts, 0, tcb)
            attention(0, 0, fuse_vproj=True)
            attention(0, 1, filler=proj_qk_gen(wk_sb, bk_sb, kts, 1))
            attention(0, 2, filler=proj_qk_gen(wq_sb, bq_sb, qts, 1))
            attention(0, 3)
            _psP.close()
            with tc.tile_pool(name="ps_y", bufs=2, space="PSUM") as ps_y:
                # out-projection for q-block qb fills the next attention
                # block's exp-paced PE idle.
                attention(1, 0)
                attention(1, 1, filler=outproj_gen(range(0, 4), ps_y), filler_start=6)
                attention(1, 2, filler=outproj_gen(range(4, 8), ps_y), filler_start=6)
                attention(1, 3, filler=outproj_gen(range(8, 12), ps_y), filler_start=6)
                for _ in outproj_gen(range(12, NT), ps_y):
                    pass

    nc.compile()
    return nc


def _get_nc():
    if "nc" not in _CACHE:
        _CACHE["nc"] = _build()
    return _CACHE["nc"]


def kernel(x, W_qkv, b_qkv, W_out, b_out):
    x = np.asarray(x, dtype=np.float32)
    W_qkv16 = np.asarray(W_qkv, dtype=np.float32).astype(np.float16)
    b_qkv = np.asarray(b_qkv, dtype=np.float32)
    W_out16 = np.asarray(W_out, dtype=np.float32).astype(np.float16)
    b_out = np.asarray(b_out, dtype=np.float32)

    nc = _get_nc()

    def pack_w(w2d):  # [K, C] -> [P, K//P, C]
        return np.ascontiguousarray(
            w2d.reshape(w2d.shape[0] // P, P, -1).transpose(1, 0, 2))

    xt16s = [np.ascontiguousarray(
        x[b].astype(np.float16).reshape(NQB, QW, DO, P).transpose(0, 3, 2, 1))
        for b in range(B)]
    in_maps = []
    for core in range(NCORES):
        b = core // 4
        c0 = (core % 4) * GH
        in_maps.append({
            "xt16": xt16s[b],
            "wq": pack_w(W_qkv16[:, c0:c0 + GH]),
            "wk": pack_w(W_qkv16[:, D + c0:D + c0 + GH]),
            "wv": pack_w(W_qkv16[:, 2 * D + c0:2 * D + c0 + GH]),
            "wo": pack_w(W_out16[c0:c0 + GH, :]),
            "bq": np.ascontiguousarray(b_qkv[c0:c0 + GH].reshape(2, P).T),
            "bk": np.ascontiguousarray(b_qkv[D + c0:D + c0 + GH].reshape(2, P).T),
            "bv": np.ascontiguousarray(b_qkv[2 * D + c0:2 * D + c0 + GH]),
        })

    import os
    trace = bool(int(os.environ.get("BASS_KERNEL_TRACE", "0")))
    res = run_bass_kernel_spmd(nc, in_maps, list(range(NCORES)), trace=trace)
    _CACHE["last_result"] = res

    parts = [res.results[i]["y"] for i in range(NCORES)]
    out = np.empty((B, L, D), dtype=np.float32)
    out[0] = parts[0] + parts[1] + parts[2] + parts[3]
    out[1] = parts[4] + parts[5] + parts[6] + parts[7]
    out += b_out
    return out


# revision 52
# speedup vs baseline: 1.1010x; 1.0056x over previous
"""Multi-head self-attention (B=2, L=2048, D=1024, H=16) on 8 Trainium2 cores.

Sharding: batch x head-group. Core c handles batch c//4 and heads
(c%4)*4 .. (c%4)*4+4. Each core computes Q/K/V projections for its 4 heads,
attention, and a partial out-projection (row-slice of W_out); the host sums
the 4 partials per batch and adds b_out.

The device pipeline runs in fp16 (fp32 PSUM accumulation everywhere): fp16
matmuls stream at 1 cycle/row on the PE vs 2 for fp32r. All inputs are
pre-packed on the host into the exact device layouts (x pre-transposed,
partition-major, multi-KB contiguous per-partition lines so the input load
runs at full HBM bandwidth) and streamed on one HWDGE queue in critical-path
order. The scalar queue carries no DMAs - exp ACTIVATEs would queue behind
them.

Device layouts (per core):
  XT  4x [128d, 8do, 512t]  x[b]^T token-block chunks
  QT/KT [128f, 2048t] per head-pair (2 heads x 64 features on partitions)
  V   [128t, 16mt, 4h, 65]  per-token values; col 64 is ones so the attention
                            matmul also emits softmax denominators
  OTn [128hd, 2pair, 2048t] normalized attention output, pair-stacked (odd
                            head shifted to partitions 64:128 via SBUF DMA)

Attention per (pair, q-block of 512), streaming 16 k-tiles:
  S^T(h0), S^T(h1) -> one 2-bank PSUM tile; one exp ACTIVATE over both banks
  (scale=1/8 folded in; logits are O(1) so no max-subtraction is needed)
  -> E [128, 2, 512] fp16; OT_h += [V_h|1].T @ E_h (PSUM accumulation).
The OT accumulators are copied to SBUF immediately (releasing their PSUM
banks) and normalized there: r = 1/OT[64] via reciprocal_approx_fast at
partition 0, gpsimd partition-broadcast, OTn = OT[:64] * r.

The attention stream is ScalarE(exp)-paced, and each engine's instruction
stream executes in order, so scheduling is about keeping the exp stream fed:
S^T matmuls lead, OT matmuls lag 4 k-tiles (they may wait on accumulator
evacuation), and all other PE work (V projection, pair-1 Q/K projections,
out-projection) is spliced between exp and the lagged OTs as fine-grained
generator "fillers" that run in the PE's exp-wait windows. PE warmup matmuls
at the start un-throttle the HAM clock gate during the input load.
PSUM: 4 banks S^T (double-buffered 2-bank tiles) + 2 banks OT accumulators +
2 banks for projection/out-projection accumulators.
"""

import sys

for _p in ("/opt/trn_rl_repo", "/opt/pypackages"):
    if _p not in sys.path:
        sys.path.insert(0, _p)

import numpy as np

import concourse.bacc as bacc
import concourse.mybir as mybir
import concourse.tile as tile
from concourse.bass_utils import run_bass_kernel_spmd

F32 = mybir.dt.float32
F16 = mybir.dt.float16
AF = mybir.ActivationFunctionType

B, L, D = 2, 2048, 1024
NH, HD = 16, 64
P = 128
NCORES = 8
NHL = 4                 # heads per core
GH = NHL * HD           # 256: per-core feature-group width
DO = D // P             # 8 contraction subtiles
NT = L // P             # 16 token tiles
NQB = 4                 # q blocks
QW = L // NQB           # 512
SCALE = float(HD) ** -0.5

_CACHE = {}


def _build():
    nc = bacc.Bacc("TRN2", target_bir_lowering=False, debug=False)

    # all inputs arrive pre-packed in device layout (partition-major with
    # multi-KB contiguous per-partition lines -> full DMA bandwidth)
    xt16 = nc.dram_tensor("xt16", [NQB, P, DO, QW], F16, kind="ExternalInput")
    wq = nc.dram_tensor("wq", [P, DO, GH], F16, kind="ExternalInput")
    wk = nc.dram_tensor("wk", [P, DO, GH], F16, kind="ExternalInput")
    wv = nc.dram_tensor("wv", [P, DO, GH], F16, kind="ExternalInput")
    wo = nc.dram_tensor("wo", [P, 2, D], F16, kind="ExternalInput")
    bq = nc.dram_tensor("bq", [P, 2], F32, kind="ExternalInput")
    bk = nc.dram_tensor("bk", [P, 2], F32, kind="ExternalInput")
    bv = nc.dram_tensor("bv", [GH], F32, kind="ExternalInput")
    y = nc.dram_tensor("y", [L, D], F32, kind="ExternalOutput")

    with tile.TileContext(nc) as tc:
        with tc.tile_pool(name="persist", bufs=1) as persist, \
             tc.tile_pool(name="wpool", bufs=1) as wpool, \
             tc.tile_pool(name="xt_pool", bufs=1) as xt_pool, \
             tc.tile_pool(name="epool", bufs=18) as epool, \
             tc.tile_pool(name="npool", bufs=4) as npool, \
             tc.tile_pool(name="ypool", bufs=3) as ypool, \
             tc.tile_pool(name="ps_s", bufs=2, space="PSUM") as ps_s, \
             tc.tile_pool(name="ps_o", bufs=1, space="PSUM") as ps_o:
            from contextlib import ExitStack
            _psP = ExitStack()
            ps_p = _psP.enter_context(tc.tile_pool(name="ps_p", bufs=2, space="PSUM"))

            # ---- PE warmup: ~10us of zero matmuls so HAM un-throttles while
            # the input DMAs stream in.
            wz = persist.tile([P, QW], F16)
            nc.vector.memset(wz[:], 0.0)
            pwm = ps_p.tile([P, QW], F32, tag="pj", name="pwm")
            for _ in range(14):
                nc.tensor.matmul(pwm[:], wz[:, 0:P], wz[:], start=True, stop=True)

            qt = persist.tile([P, 2, L], F16)
            kt = persist.tile([P, 2, L], F16)
            qts = [qt[:, i, :] for i in range(2)]
            kts = [kt[:, i, :] for i in range(2)]
            v = persist.tile([P, NT, NHL, HD + 1], F16)
            otn = persist.tile([P, 2, L], F16)

            # x arrives pre-transposed and pre-packed from the host.
            xts = [xt_pool.tile([P, DO, QW], F16, name=f"xt{i}", tag=f"xt{i}")
                   for i in range(NQB)]
            def xt_chunk(q, tcb):
                q.dma_start(xts[tcb][:], xt16[tcb])

            wk_sb = wpool.tile([P, DO, GH], F16)
            wq_sb = wpool.tile([P, DO, GH], F16)
            wv_sb = wpool.tile([P, DO, GH], F16)
            wo_sb = wpool.tile([P, 2, D], F16)
            bq_sb = persist.tile([P, 2], F32)
            bk_sb = persist.tile([P, 2], F32)
            bv_row = persist.tile([1, GH], F32)
            # NOTE: never issue DMAs on the scalar queue — the exp ACTIVATEs
            # would queue behind them (strict FIFO) and the whole attention
            # stream is exp-paced. Bulk loads go on sync; gpsimd (SWDGE)
            # carries the later chunks in parallel.
            # single queue => in-order service at full HBM bandwidth, in
            # critical-path order (the load is HBM-bound at ~350GB/s).
            nc.sync.dma_start(wk_sb[:], wk[:])
            nc.sync.dma_start(wq_sb[:], wq[:])
            xt_chunk(nc.sync, 0)
            xt_chunk(nc.sync, 1)
            xt_chunk(nc.sync, 2)
            xt_chunk(nc.sync, 3)
            nc.sync.dma_start(wv_sb[:], wv[:])
            nc.sync.dma_start(wo_sb[:], wo[:])
            nc.gpsimd.dma_start(bq_sb[:], bq[:])
            nc.gpsimd.dma_start(bk_sb[:], bk[:])
            nc.gpsimd.dma_start(bv_row[:], bv[None, :])
            bv_sb = persist.tile([P, GH], F32)
            nc.gpsimd.partition_broadcast(bv_sb[:], bv_row[:])
            # ones column for the denominator rows
            ones_sb = persist.tile([P, 1], F32)
            nc.vector.memset(ones_sb[:], 1.0)
            nc.vector.tensor_copy(
                v[:, :, :, HD], ones_sb[:, 0:1, None].to_broadcast((P, NT, NHL)))

            def proj_v(mt):
                pv = ps_p.tile([P, QW], F32, tag="pj", name="pv")
                for do in range(DO):
                    nc.tensor.matmul(
                        pv[:, 0:GH],
                        xts[mt // 4][:, do, (mt % 4) * P:(mt % 4 + 1) * P],
                        wv_sb[:, do, :],
                        start=(do == 0), stop=(do == DO - 1),
                    )
                nc.vector.tensor_add(
                    out=v[:, mt, :, 0:HD],
                    in0=pv[:, 0:GH].rearrange("p (h c) -> p h c", h=NHL),
                    in1=bv_sb[:].rearrange("p (h c) -> p h c", h=NHL),
                )

            def proj_qk(w_sb, b_sb, dst, pr, tcb):
                pj = ps_p.tile([P, QW], F32, tag="pj", name="pj")
                for do in range(DO):
                    nc.tensor.matmul(
                        pj[:],
                        w_sb[:, do, pr * P:(pr + 1) * P],
                        xts[tcb][:, do, :],
                        start=(do == 0), stop=(do == DO - 1),
                    )
                nc.vector.tensor_scalar_add(
                    dst[pr][:, tcb * QW:(tcb + 1) * QW], pj[:], b_sb[:, pr:pr + 1])

            def proj_qk_gen(w_sb, b_sb, dst, pr):
                # generator form: 2 matmuls per step, one step per attention
                # k-tile, so the projection never delays the next S^T pair.
                for tcb in range(NQB):
                    pj = ps_p.tile([P, QW], F32, tag="pj", name="pj")
                    for do in range(DO):
                        nc.tensor.matmul(
                            pj[:],
                            w_sb[:, do, pr * P:(pr + 1) * P],
                            xts[tcb][:, do, :],
                            start=(do == 0), stop=(do == DO - 1),
                        )
                        if do % 2 == 1:
                            yield
                    nc.vector.tensor_scalar_add(
                        dst[pr][:, tcb * QW:(tcb + 1) * QW], pj[:], b_sb[:, pr:pr + 1])

            def outproj_gen(mts, ps_y):
                for mt in mts:
                    y_sb = ypool.tile([P, D], F32, name="y_sb")
                    for nh in range(2):
                        py = ps_y.tile([P, QW], F32, name="py")
                        for pr in range(2):
                            nc.tensor.matmul(
                                py[:],
                                otn[:, pr, mt * P:(mt + 1) * P],
                                wo_sb[:, pr, nh * QW:(nh + 1) * QW],
                                start=(pr == 0), stop=(pr == 1),
                            )
                        nc.vector.tensor_copy(y_sb[:, nh * QW:(nh + 1) * QW], py[:])
                        yield
                    nc.sync.dma_start(y[mt * P:(mt + 1) * P, :], y_sb[:])

            def attention(pr, qb, fuse_vproj=False, filler=None, filler_start=0):
                qs = slice(qb * QW, (qb + 1) * QW)
                po = [ps_o.tile([HD + 1, QW], F32, tag=f"po{i}", name=f"po{i}")
                      for i in range(2)]
                OT_LAG = 6
                es = {}

                def emit_ot(kt):
                    for hi in range(2):
                        nc.tensor.matmul(
                            po[hi][:],
                            v[:, kt, 2 * pr + hi, :],
                            es[kt][:, hi, :],
                            start=(kt == 0), stop=(kt == NT - 1),
                        )

                for ktile in range(NT):
                    pS = ps_s.tile([P, 2, QW], F32, name="pS")
                    for hi in range(2):
                        lo = hi * HD
                        nc.tensor.matmul(
                            pS[:, hi, :],
                            kts[pr][lo:lo + HD, ktile * P:(ktile + 1) * P],
                            qts[pr][lo:lo + HD, qs],
                            start=True, stop=True,
                        )
                    if fuse_vproj:
                        # v[ktile] is first read by this block's OT matmuls;
                        # emitting its projection here (below the S^T matmuls
                        # in priority) lets the exp stream start ~30us earlier.
                        proj_v(ktile)
                    es[ktile] = epool.tile([P, 2, QW], F16, name="e_sb")
                    nc.scalar.activation(es[ktile][:], pS[:], AF.Exp, scale=SCALE)
                    # filler sits between exp and the (lagged) OT pair: the
                    # in-order PE stream is waiting on exp here, so this is
                    # free PE time. filler_start>0 skips early k-tiles when the
                    # first piece depends on the previous block's normalize.
                    if filler is not None and ktile >= filler_start:
                        next(filler, None)
                    # OTs lag 2 k-tiles: a new block's first STs (which feed the
                    # exp stream) are never queued behind the OT matmuls that
                    # wait on the previous block's accumulator evacuation.
                    if ktile >= OT_LAG:
                        emit_ot(ktile - OT_LAG)
                for kt in range(NT - OT_LAG, NT):
                    emit_ot(kt)
                if filler is not None:
                    for _ in filler:
                        pass
                for hi in range(2):
                    # Evacuate the accumulator to SBUF right away (frees its
                    # PSUM bank for the next block); normalize from the copy.
                    ou = npool.tile([HD + 1, QW], F32, tag="ou", name="ou")
                    nc.vector.tensor_copy(ou[:], po[hi][:])
                    s0 = npool.tile([1, QW], F32, tag="s0", name="s0")
                    nc.sync.dma_start(s0[:], ou[HD:HD + 1, :])
                    r0 = npool.tile([1, QW], F32, tag="r0", name="r0")
                    nc.vector.reciprocal_approx_fast(r0[:], s0[:])
                    rb = npool.tile([HD, QW], F32, tag="rb", name="rb")
                    nc.gpsimd.partition_broadcast(rb[:], r0[:])
                    if hi == 0:
                        nc.vector.tensor_mul(
                            out=otn[0:HD, pr, qs], in0=ou[0:HD, :], in1=rb[:])
                    else:
                        tmp = npool.tile([HD, QW], F16, tag="tmp", name="tmp")
                        nc.vector.tensor_mul(out=tmp[:], in0=ou[0:HD, :], in1=rb[:])
                        nc.sync.dma_start(otn[HD:P, pr, qs], tmp[:])

            def outproj(mt, ps_y):
                y_sb = ypool.tile([P, D], F32, name="y_sb")
                for nh in range(2):
                    py = ps_y.tile([P, QW], F32, name="py")
                    for pr in range(2):
                        nc.tensor.matmul(
                            py[:],
                            otn[:, pr, mt * P:(mt + 1) * P],
                            wo_sb[:, pr, nh * QW:(nh + 1) * QW],
                            start=(pr == 0), stop=(pr == 1),
                        )
                    nc.vector.tensor_copy(y_sb[:, nh * QW:(nh + 1) * QW], py[:])
                nc.sync.dma_start(y[mt * P:(mt + 1) * P, :], y_sb[:])

            # ---- schedule: pair-0 K/Q first so the exp stream starts early
            # (S^T needs only K/Q; the deep E pool lets exp run ahead of the
            # V-dependent OT matmuls while the PE catches up on V). Pair-1
            # projections are emitted between pair-0 attention blocks to fill
            # the exp stream's PE idle. NOTE: Tile dependencies follow
            # emission order, so every tensor must be emitted-written before
            # an emitted-read (no read-before-write reordering).
            for tcb in range(NQB):
                proj_qk(wk_sb, bk_sb, kts, 0, tcb)
                proj_qk(wq_sb, bq_sb, qts, 0, tcb)
            attention(0, 0, fuse_vproj=True)
            attention(0, 1, filler=proj_qk_gen(wk_sb, bk_sb, kts, 1))
            attention(0, 2, filler=proj_qk_gen(wq_sb, bq_sb, qts, 1))
            attention(0, 3)
            _psP.close()
            with tc.tile_pool(name="ps_y", bufs=2, space="PSUM") as ps_y:
                # out-projection for q-block qb fills the next attention
                # block's exp-paced PE idle.
                attention(1, 0)
                attention(1, 1, filler=outproj_gen(range(0, 4), ps_y), filler_start=6)
                attention(1, 2, filler=outproj_gen(range(4, 8), ps_y), filler_start=6)
                attention(1, 3, filler=outproj_gen(range(8, 12), ps_y), filler_start=6)
                for _ in outproj_gen(range(12, NT), ps_y):
                    pass

    nc.compile()
    return nc


def _get_nc():
    if "nc" not in _CACHE:
        _CACHE["nc"] = _build()
    return _CACHE["nc"]


def kernel(x, W_qkv, b_qkv, W_out, b_out):
    x = np.asarray(x, dtype=np.float32)
    W_qkv16 = np.asarray(W_qkv, dtype=np.float32).astype(np.float16)
    b_qkv = np.asarray(b_qkv, dtype=np.float32)
    W_out16 = np.asarray(W_out, dtype=np.float32).astype(np.float16)
    b_out = np.asarray(b_out, dtype=np.float32)

    nc = _get_nc()

    def pack_w(w2d):  # [K, C] -> [P, K//P, C]
        return np.ascontiguousarray(
            w2d.reshape(w2d.shape[0] // P, P, -1).transpose(1, 0, 2))

    xt16s = [np.ascontiguousarray(
        x[b].astype(np.float16).reshape(NQB, QW, DO, P).transpose(0, 3, 2, 1))
        for b in range(B)]
    in_maps = []
    for core in range(NCORES):
        b = core // 4
        c0 = (core % 4) * GH
        in_maps.append({
            "xt16": xt16s[b],
            "wq": pack_w(W_qkv16[:, c0:c0 + GH]),
            "wk": pack_w(W_qkv16[:, D + c0:D + c0 + GH]),
            "wv": pack_w(W_qkv16[:, 2 * D + c0:2 * D + c0 + GH]),
            "wo": pack_w(W_out16[c0:c0 + GH, :]),
            "bq": np.ascontiguousarray(b_qkv[c0:c0 + GH].reshape(2, P).T),
            "bk": np.ascontiguousarray(b_qkv[D + c0:D + c0 + GH].reshape(2, P).T),
            "bv": np.ascontiguousarray(b_qkv[2 * D + c0:2 * D + c0 + GH]),
        })

    import os
    trace = bool(int(os.environ.get("BASS_KERNEL_TRACE", "0")))
    res = run_bass_kernel_spmd(nc, in_maps, list(range(NCORES)), trace=trace)
    _CACHE["last_result"] = res

    parts = [res.results[i]["y"] for i in range(NCORES)]
    out = np.empty((B, L, D), dtype=np.float32)
    out[0] = parts[0] + parts[1] + parts[2] + parts[3]
    out[1] = parts[4] + parts[5] + parts[6] + parts[7]
    out += b_out
    return out
